# revision 1
# baseline (speedup 1.0000x reference)
"""Trainium2 Bass kernel for nn_CorefModel (GNN message passing, 8 NeuronCores).

Sharding: constituents 2048/core, tokens 1024/core (row shards). Per-iteration
node-feature tables (hf, hb, hk, tok_cons) are produced by per-core GEMMs,
AllGathered into a full per-core HBM copy, and edge gathers (dma_gather) read
full rows from the local copy. GAT edge aggregation runs as one-hot scatter
matmuls on the TensorEngine over destination-sorted edge chunks; the softmax
denominator is accumulated by the same one-hot against exp values. Attention
head scalars are folded into the table GEMMs (W @ blockdiag(a) extra columns).
Compute dtype bf16 with fp32 PSUM accumulation; final blend in fp32.
"""
import sys, os
sys.path.insert(0, '/opt/trn_rl_repo')
import math
import numpy as np
import ml_dtypes

import concourse.bass as bass
from concourse import bacc
import concourse.tile as tile
from concourse import mybir
from concourse.bass_utils import run_bass_kernel_spmd

BF = ml_dtypes.bfloat16
F32 = np.float32

H = 8
NT, NC = 8192, 16384
DH, DCH = 768, 128
D = 2 * DH + DCH          # 1664
NCORES = 8
NCL, NTL = NC // NCORES, NT // NCORES     # 2048, 1024
WCC, WCT = NCL // 128, NTL // 128         # 16, 8
SWF = 1792                 # cc table row stride (bf16 elems, 3584B)
SWK = 896                  # hk table row stride (1792B)

f32 = mybir.dt.float32
bf16 = mybir.dt.bfloat16
i16 = mybir.dt.int16
AF = mybir.ActivationFunctionType
OP = mybir.AluOpType


# ---------------------------------------------------------------- host prep --

def _bdiag(a):
    h, dh = a.shape
    m = np.zeros((h * dh, h), F32)
    for i in range(h):
        m[i * dh:(i + 1) * dh, i] = a[i]
    return m


def _idx_plane(idxs):
    """[128, n/16] int16: idx i at (i%16, i//16), replicated across 8 Q7 groups."""
    n = len(idxs)
    plane = np.zeros((128, n // 16), np.int16)
    plane[np.arange(n) % 16, np.arange(n) // 16] = idxs
    for g in range(1, 8):
        plane[16 * g:16 * (g + 1)] = plane[:16]
    return plane


def _prep_edges(srcs, dsts, wins):
    """Per edge set: sort by dst, split into per-(core, 128-dst-window) groups.
    Returns uniform chunks/window C and per-core index/dstlocal arrays."""
    nsets = len(srcs)
    percw = [[[None] * wins for _ in range(nsets)] for _ in range(NCORES)]
    maxe = 0
    for s in range(nsets):
        src, dst = srcs[s], dsts[s]
        gwin = dst // 128
        order = np.argsort(gwin, kind='stable')
        src_s, dst_s, gwin_s = src[order], dst[order], gwin[order]
        bounds = np.searchsorted(gwin_s, np.arange(NCORES * wins + 1))
        for c in range(NCORES):
            for w in range(wins):
                g = c * wins + w
                lo, hi = bounds[g], bounds[g + 1]
                maxe = max(maxe, hi - lo)
                percw[c][s][w] = (src_s[lo:hi], dst_s[lo:hi] % 128)
    C = max(1, math.ceil(maxe / 128))
    E = C * 128
    out = []
    for c in range(NCORES):
        idxp = np.zeros((128, nsets * wins * (E // 16)), np.int16)
        dl = np.full((128, nsets * wins * C), -1.0, F32)
        dlr = np.full((128, nsets * wins * E), -1.0, F32)
        for s in range(nsets):
            for w in range(wins):
                src_e, dloc = percw[c][s][w]
                k = len(src_e)
                blk = s * wins + w
                sp = np.zeros(E, np.int16); sp[:k] = src_e.astype(np.int16)
                dp = np.full(E, -1.0, F32); dp[:k] = dloc.astype(F32)
                idxp[:, blk * (E // 16):(blk + 1) * (E // 16)] = _idx_plane(sp)
                dl[:, blk * C:(blk + 1) * C] = dp.reshape(C, 128).T
                dlr[:, blk * E:(blk + 1) * E] = dp[None, :]
        out.append(dict(idx=idxp, dl=dl.astype(BF), dlr=dlr.astype(BF)))
    return C, out


def _host_prep(inp):
    tok = np.asarray(inp['token_embeddings'], F32)
    starts = np.asarray(inp['constituent_starts']).astype(np.int64)
    ends = np.asarray(inp['constituent_ends']).astype(np.int64)
    labels = np.asarray(inp['constituent_labels']).astype(np.int64)
    label_emb = np.asarray(inp['cons_type_table'], F32)[labels]        # [NC, 128]

    Wf = np.asarray(inp['Wf'], F32); Wb = np.asarray(inp['Wb'], F32)
    Wk = np.asarray(inp['Wk'], F32); Wq = np.asarray(inp['Wq'], F32)
    Wf_ext = np.concatenate([Wf, Wf @ _bdiag(np.asarray(inp['af_s'], F32)),
                             Wf @ _bdiag(np.asarray(inp['af_d'], F32))], 1).astype(BF)
    Wb_ext = np.concatenate([Wb, Wb @ _bdiag(np.asarray(inp['ab_s'], F32)),
                             Wb @ _bdiag(np.asarray(inp['ab_d'], F32))], 1).astype(BF)
    Wk_ext = np.concatenate([Wk, Wk @ _bdiag(np.asarray(inp['act_s'], F32))], 1).astype(BF)
    Wq_ad = (Wq @ _bdiag(np.asarray(inp['act_d'], F32))).astype(BF)    # [768, 8]
    w1 = np.asarray(inp['attn_w1'], F32).astype(BF)                    # [1664, 1024]
    w2c = np.asarray(inp['attn_w2'], F32).reshape(8, 128).T.copy().astype(BF)   # [128, 8]
    b1c = np.asarray(inp['attn_b1'], F32).reshape(8, 128).T.copy()     # [128, 8] f32
    fuse_w = np.asarray(inp['fuse_w'], F32).astype(BF)                 # [1536, 768]
    fb_row = np.asarray(inp['fuse_b'], F32).reshape(1, DH).astype(BF)

    cons = np.concatenate([tok[starts], tok[ends], label_emb], 1)      # [NC, 1664] f32

    cc_src = np.asarray(inp['cc_src']); cc_dst = np.asarray(inp['cc_dst'])
    ct_src = np.asarray(inp['ct_src']); ct_dst = np.asarray(inp['ct_dst'])

    def _remap_half(g):
        # table rows after two half-shard AllGathers: half A = local rows 0:1024
        # of every rank (global rows 0:8192), half B = local rows 1024:2048.
        r = g // NCL
        l = g % NCL
        return np.where(l < NTL, r * NTL + l, NC // 2 + r * NTL + (l - NTL))
    cc_src = _remap_half(cc_src)
    ct_src = _remap_half(ct_src)
    C_CC, cc_meta = _prep_edges([cc_src[s] for s in range(4)], [cc_dst[s] for s in range(4)], WCC)
    C_CT, ct_meta = _prep_edges([ct_src], [ct_dst], WCT)
    if os.environ.get("DBG_CT_FAKE") == "1":
        for c in range(NCORES):
            ct_meta[c]['idx'][:] = 0
            ct_meta[c]['dl'][:] = -1.0
            ct_meta[c]['dlr'][:] = -1.0
    if os.environ.get("DBG_CT_C"):
        cap = int(os.environ["DBG_CT_C"])
        if C_CT > cap:
            C_CT = cap
            E = cap * 128
            for c in range(NCORES):
                full = WCT * (17 * 128)
                # rebuild capped arrays by truncating per-window blocks is complex; just slice per blk
            # simple approach: rebuild via _prep_edges on truncated edges is wrong; instead
            # truncate arrays blockwise below
            for c in range(NCORES):
                idxp = ct_meta[c]['idx']; dl = ct_meta[c]['dl']; dlr = ct_meta[c]['dlr']
                ob = idxp.reshape(128, WCT, -1)[:, :, :E // 16].reshape(128, -1)
                odl = dl.reshape(128, WCT, -1)[:, :, :cap].reshape(128, -1)
                odlr = dlr.reshape(128, WCT, -1)[:, :, :E].reshape(128, -1)
                ct_meta[c] = dict(idx=ob.copy(), dl=odl.copy(), dlr=odlr.copy())

    CMAX = max(C_CC, C_CT)
    iota_rep = np.tile(np.arange(128, dtype=F32), CMAX)[None, :].repeat(128, 0).astype(BF)
    iota_col = np.arange(128, dtype=F32)[:, None]
    ident = np.eye(128, dtype=F32).astype(BF)
    ones1 = np.ones((1, 128), F32).astype(BF)

    in_maps = []
    for c in range(NCORES):
        csl = slice(c * NCL, (c + 1) * NCL)
        tsl = slice(c * NTL, (c + 1) * NTL)
        m = dict(
            consT0=np.ascontiguousarray(cons[csl].T).astype(BF),       # [1664, 2048]
            labelT=np.ascontiguousarray(label_emb[csl].T).astype(BF),  # [128, 2048]
            tokT=np.ascontiguousarray(tok[tsl].T).astype(BF),          # [768, 1024]
            tok_f32=np.ascontiguousarray(tok[tsl]),                    # [1024, 768]
            idx_starts=_idx_plane(starts[csl].astype(np.int16)),       # [128, 128]
            idx_ends=_idx_plane(ends[csl].astype(np.int16)),
            wf_ext=Wf_ext, wb_ext=Wb_ext, wk_ext=Wk_ext, wq_ad=Wq_ad,
            w1=w1, w2c=w2c, b1c=b1c, fuse_w=fuse_w, fb_row=fb_row,
            iota_rep=iota_rep, iota_col=iota_col, ident=ident, ones1=ones1,
            idx_cc=cc_meta[c]['idx'], dl_cc=cc_meta[c]['dl'], dlr_cc=cc_meta[c]['dlr'],
            idx_ct=ct_meta[c]['idx'], dl_ct=ct_meta[c]['dl'], dlr_ct=ct_meta[c]['dlr'],
        )
        in_maps.append(m)
    return in_maps, C_CC, C_CT


# ------------------------------------------------------------- device build --

def _build_nc(C_CC, C_CT):
    ECC = C_CC * 128
    ECT = C_CT * 128
    CMAX = max(C_CC, C_CT)
    nc = bacc.Bacc("TRN2", num_devices=NCORES)

    def ein(name, shape, dt_):
        return nc.dram_tensor(name, shape, dt_, kind="ExternalInput")

    consT0 = ein("consT0", [D, NCL], bf16)
    labelT = ein("labelT", [128, NCL], bf16)
    tokT = ein("tokT", [DH, NTL], bf16)
    tok_f32_d = ein("tok_f32", [NTL, DH], f32)
    idx_starts = ein("idx_starts", [128, NCL // 16], i16)
    idx_ends = ein("idx_ends", [128, NCL // 16], i16)
    wf_ext = ein("wf_ext", [D, 1680], bf16)
    wb_ext = ein("wb_ext", [D, 1680], bf16)
    wk_ext = ein("wk_ext", [D, 776], bf16)
    wq_ad = ein("wq_ad", [DH, 8], bf16)
    w1_d = ein("w1", [D, 1024], bf16)
    w2c_d = ein("w2c", [128, 8], bf16)
    b1c_d = ein("b1c", [128, 8], f32)
    fuse_w_d = ein("fuse_w", [2 * DH, DH], bf16)
    fb_row_d = ein("fb_row", [1, DH], bf16)
    iota_rep_d = ein("iota_rep", [128, CMAX * 128], bf16)
    iota_col_d = ein("iota_col", [128, 1], f32)
    ident_d = ein("ident", [128, 128], bf16)
    ones1_d = ein("ones1", [1, 128], bf16)
    idx_cc_d = ein("idx_cc", [128, 4 * WCC * (ECC // 16)], i16)
    dl_cc_d = ein("dl_cc", [128, 4 * WCC * C_CC], bf16)
    dlr_cc_d = ein("dlr_cc", [128, 4 * WCC * ECC], bf16)
    idx_ct_d = ein("idx_ct", [128, WCT * (ECT // 16)], i16)
    dl_ct_d = ein("dl_ct", [128, WCT * C_CT], bf16)
    dlr_ct_d = ein("dlr_ct", [128, WCT * ECT], bf16)

    out_d = nc.dram_tensor("out", [NTL, DH], f32, kind="ExternalOutput")

    tf_in = nc.dram_tensor("tf_in", [NCL, SWF], bf16)
    tb_in = nc.dram_tensor("tb_in", [NCL, SWF], bf16)
    tk_in = nc.dram_tensor("tk_in", [NCL, SWK], bf16)
    tt_in = nc.dram_tensor("tt_in", [NTL, DH], bf16)
    tf_tab = nc.dram_tensor("tf_tab", [NC, SWF], bf16, addr_space="Shared")
    tb_tab = nc.dram_tensor("tb_tab", [NC, SWF], bf16, addr_space="Shared")
    tk_tab = nc.dram_tensor("tk_tab", [NC, SWK], bf16, addr_space="Shared")
    tt_tab = nc.dram_tensor("tt_tab", [NT, DH], bf16, addr_space="Shared")
    sT_dram = [nc.dram_tensor(f"sT{j}", [13, 128, NCL], bf16) for j in range(2)]
    ord_dram = nc.dram_tensor("ord0", [NCL, D], bf16)
    tk0_dram = nc.dram_tensor("tk0", [NCL, 776], bf16)
    tk1_dram = nc.dram_tensor("tk1", [NCL, 776], bf16)
    tcf_dram = nc.dram_tensor("tcf", [NTL, DH], f32)

    RG = [list(range(NCORES))]

    with tile.TileContext(nc) as tc:
      with tc.tile_pool(name="const", bufs=1) as cp, \
           tc.tile_pool(name="keep1", bufs=1) as kp1, \
           tc.tile_pool(name="keep2", bufs=2) as kp2:
        def cload(name, dram, shape, dt_):
            t = cp.tile(shape, dt_, name=name)
            nc.sync.dma_start(out=t[:], in_=dram[:])
            return t
        iota_rep_t = cload("iota_rep", iota_rep_d, [128, CMAX * 128], bf16)
        iota_col_t = cload("iota_col", iota_col_d, [128, 1], f32)
        ident_t = cload("ident", ident_d, [128, 128], bf16)
        ones1_t = cload("ones1", ones1_d, [1, 128], bf16)
        idx_cc_t = cload("idx_cc", idx_cc_d, [128, 4 * WCC * (ECC // 16)], i16)
        dl_cc_t = cload("dl_cc", dl_cc_d, [128, 4 * WCC * C_CC], bf16)
        idx_ct_t = cload("idx_ct", idx_ct_d, [128, WCT * (ECT // 16)], i16)
        dl_ct_t = cload("dl_ct", dl_ct_d, [128, WCT * C_CT], bf16)
        w2c_t = cload("w2c", w2c_d, [128, 8], bf16)
        b1c_t = cload("b1c", b1c_d, [128, 8], f32)

        def gemm_block(ps, lhsT_fn, nkt, w_t, nchunks, stg, tag):
            for (n0, nw) in nchunks:
                pt = ps.tile([128, 512], f32, name=f"gp{tag}")
                for k in range(nkt):
                    nc.tensor.matmul(out=pt[:, :nw], lhsT=lhsT_fn(k),
                                     rhs=w_t[:, k, n0:n0 + nw],
                                     start=(k == 0), stop=(k == nkt - 1))
                nc.scalar.activation(out=stg[:, n0:n0 + nw], in_=pt[:, :nw], func=AF.Copy)

        def gat_pass(sb, ps_pv, ps_sm, ps_pd, tab, stride, idx_t, dl_t, dlr_d, sd_tile,
                     wins, C, dfeat, blk0, writeback, wb_final=lambda w, sb: None, tag=""):
            E = C * 128
            dh = dfeat // 8
            for w in range(wins):
                blk = blk0 + w
                g = sb.tile([128, C, stride], bf16, name=f"gw{tag}")
                for c0 in range(0, C, 5):
                    cc = min(5, C - c0)
                    ioff = blk * (E // 16) + c0 * 8
                    if c0 == 0:
                        nc.gpsimd.dma_gather(
                            out_ap=g[:, 0:cc, :], in_ap=tab[:],
                            idxs_ap=idx_t[:, ioff:ioff + cc * 8],
                            num_idxs=cc * 128, num_idxs_reg=cc * 128, elem_size=stride)
                    else:
                        gtmp = sb.tile([128, 5, stride], bf16, name=f"gtmp{tag}")
                        nc.gpsimd.dma_gather(
                            out_ap=gtmp[:, 0:cc, :], in_ap=tab[:],
                            idxs_ap=idx_t[:, ioff:ioff + cc * 8],
                            num_idxs=cc * 128, num_idxs_reg=cc * 128, elem_size=stride)
                        nc.vector.tensor_copy(out=g[:, c0:c0 + cc, :], in_=gtmp[:, 0:cc, :])
                dlr = sb.tile([128, E], bf16, name=f"dlr{tag}")
                nc.sync.dma_start(out=dlr[:], in_=dlr_d[:, blk * E:(blk + 1) * E])
                o01 = sb.tile([128, C, 128], bf16, name=f"o01{tag}")
                nc.vector.tensor_tensor(
                    out=o01[:],
                    in0=iota_rep_t[:, :C * 128].rearrange("p (c j) -> p c j", c=C),
                    in1=dl_t[:, blk * C:(blk + 1) * C].to_broadcast([128, C, 128]),
                    op=OP.is_equal)
                ob = sb.tile([128, C, 128], bf16, name=f"ob{tag}")
                nc.vector.tensor_scalar(
                    out=ob[:], in0=dlr[:].rearrange("p (c e) -> p c e", c=C),
                    scalar1=iota_col_t[:, :1], scalar2=None, op0=OP.is_equal)
                strip = ps_sm.tile([128, 512], f32, name=f"strip{tag}")
                for c in range(C):
                    nc.tensor.matmul(out=strip[:, c * 8:(c + 1) * 8], lhsT=ob[:, c, :],
                                     rhs=sd_tile[:, w, :], start=True, stop=False)
                    nc.tensor.matmul(out=strip[:, c * 8:(c + 1) * 8], lhsT=ident_t[:],
                                     rhs=g[:, c, dfeat:dfeat + 8], start=False, stop=True)
                lg = sb.tile([128, C * 8], f32, name=f"lg{tag}")
                nc.scalar.activation(out=lg[:], in_=strip[:, :C * 8], func=AF.Prelu, alpha=0.2)
                ex = sb.tile([128, C, 8], bf16, name=f"ex{tag}")
                nc.scalar.activation(out=ex[:].rearrange("p c e -> p (c e)"), in_=lg[:],
                                     func=AF.Exp)
                rss = []
                for c in range(C):
                    rs = sb.tile([128, 8, dh], bf16, name=f"rs{tag}")
                    nc.vector.tensor_tensor(
                        out=rs[:], in0=g[:, c, :dfeat].rearrange("p (h d) -> p h d", h=8),
                        in1=ex[:, c, :].to_broadcast([128, 8, dh]), op=OP.mult)
                    rss.append(rs)
                half = dfeat // 2
                hchunks = []
                n0 = 0
                while n0 < half:
                    nw = min(512, half - n0)
                    hchunks.append((n0, nw)); n0 += nw
                rec = None
                pvsz = 1024 if half > 512 else 512
                for hh in range(2):
                    pv = ps_pv.tile([128, pvsz], f32, name=f"pv{tag}")
                    for c in range(C):
                        rsf = rss[c][:].rearrange("p h d -> p (h d)")
                        for (n0, nw) in hchunks:
                            nc.tensor.matmul(out=pv[:, n0:n0 + nw], lhsT=o01[:, c, :],
                                             rhs=rsf[:, hh * half + n0:hh * half + n0 + nw],
                                             start=(c == 0), stop=(c == C - 1))
                        if hh == 0:
                            nc.tensor.matmul(out=strip[:, 384:392], lhsT=o01[:, c, :],
                                             rhs=ex[:, c, :], start=(c == 0), stop=(c == C - 1))
                    if hh == 0:
                        dent = sb.tile([128, 8], f32, name=f"dent{tag}")
                        nc.vector.tensor_scalar(out=dent[:], in0=strip[:, 384:392],
                                                scalar1=1e-9, scalar2=None, op0=OP.add)
                        rec = sb.tile([128, 8], f32, name=f"rec{tag}")
                        nc.vector.reciprocal(out=rec[:], in_=dent[:])
                    writeback(w, hh, pv, rec, sb)
                wb_final(w, sb)

        # ============================ iterations ============================
        PH = int(os.environ.get("PHASE_LIMIT", "99"))
        NITER = int(os.environ.get("NITER", "2"))
        tokcT_cur = None
        for it in range(NITER):
            # -------- P1+P2: consT, hf/hb tables, AllGather -----------------
            with tc.tile_pool(name=f"p2_{it}", bufs=1) as sb2, \
                 tc.tile_pool(name=f"p2s_{it}", bufs=3) as sb2s, \
                 tc.tile_pool(name=f"p2p_{it}", bufs=2, space="PSUM") as ps2:
                consT = sb2.tile([128, 13, NCL], bf16, name="consT")
                if it == 0:
                    nc.sync.dma_start(out=consT[:],
                                      in_=consT0[:].rearrange("(k p) e -> p k e", p=128))
                else:
                    ist = sb2.tile([128, NCL // 16], i16, name="ist")
                    nc.sync.dma_start(out=ist[:], in_=idx_starts[:])
                    ien = sb2.tile([128, NCL // 16], i16, name="ien")
                    nc.sync.dma_start(out=ien[:], in_=idx_ends[:])
                    for half, idxt in ((0, ist), (1, ien)):
                        for q in range(4):
                            gt = sb2s.tile([128, 6, 512], bf16, name="gtc")
                            nc.gpsimd.dma_gather(
                                out_ap=gt[:], in_ap=tt_tab[:],
                                idxs_ap=idxt[:, q * 32:(q + 1) * 32],
                                num_idxs=512, num_idxs_reg=512,
                                elem_size=DH, transpose=True)
                            nc.vector.tensor_copy(
                                out=consT[:, 6 * half:6 * half + 6, q * 512:(q + 1) * 512],
                                in_=gt[:])
                    nc.sync.dma_start(out=consT[:, 12, :], in_=labelT[:])
                sd_f = kp1.tile([128, WCC, 8], bf16, name=f"sd_f{it}")
                sd_b = kp1.tile([128, WCC, 8], bf16, name=f"sd_b{it}")
                for (wd, outd, tabd, sdt) in [(wf_ext, tf_in, tf_tab, sd_f),
                                              (wb_ext, tb_in, tb_tab, sd_b)]:
                    w_t = sb2.tile([128, 13, 1680], bf16, name="wtab")
                    nc.sync.dma_start(out=w_t[:], in_=wd[:].rearrange("(k p) n -> p k n", p=128))
                    for m in range(WCC):
                        stg = sb2s.tile([128, 1680], bf16, name="stg")
                        gemm_block(ps2, lambda k: consT[:, k, m * 128:(m + 1) * 128], 13,
                                   w_t, [(0, 512), (512, 512), (1024, 512), (1536, 144)],
                                   stg, "t")
                        nc.sync.dma_start(out=outd[m * 128:(m + 1) * 128, :1680], in_=stg[:])
                        nc.vector.tensor_copy(out=sdt[:, m, :], in_=stg[:, 1672:1680])
                        if m == WCC // 2 - 1:
                            nc.gpsimd.collective_compute(
                                "AllGather", OP.bypass, replica_groups=RG,
                                ins=[outd[0:NCL // 2, :]], outs=[tabd[0:NC // 2, :]])
                    nc.gpsimd.collective_compute(
                        "AllGather", OP.bypass, replica_groups=RG,
                        ins=[outd[NCL // 2:NCL, :]], outs=[tabd[NC // 2:NC, :]])

            # -------- P3: cc GATs -> order_j, transposed-spill to sT_dram ---
            if PH < 3: break
            with tc.tile_pool(name=f"p3_{it}", bufs=1) as sb3o, \
                 tc.tile_pool(name=f"p3s_{it}", bufs=3) as sb3, \
                 tc.tile_pool(name=f"p3pv_{it}", bufs=2, space="PSUM") as ps3a, \
                 tc.tile_pool(name=f"p3ps_{it}", bufs=2, space="PSUM") as ps3b, \
                 tc.tile_pool(name=f"p3pd_{it}", bufs=1, space="PSUM") as ps3d, \
                 tc.tile_pool(name=f"p3pt_{it}", bufs=1, space="PSUM") as ps3c:

                def mk_wb_first(ordt):
                    def wb_first(w, hh, pv, rec, sb):
                        nc.vector.tensor_tensor(
                            out=ordt[:, w, hh * 832:(hh + 1) * 832]
                                .rearrange("p (h d) -> p h d", h=4),
                            in0=pv[:, :832].rearrange("p (h d) -> p h d", h=4),
                            in1=rec[:, hh * 4:(hh + 1) * 4].to_broadcast([128, 4, 208]),
                            op=OP.mult)
                    return wb_first

                def mk_wb_add(ordt, j):
                    def wb_add(w, hh, pv, rec, sb):
                        t = sb.tile([128, 4, 208], bf16, name="tadd")
                        nc.vector.tensor_tensor(
                            out=t[:], in0=pv[:, :832].rearrange("p (h d) -> p h d", h=4),
                            in1=rec[:, hh * 4:(hh + 1) * 4].to_broadcast([128, 4, 208]),
                            op=OP.mult)
                        nc.vector.tensor_tensor(
                            out=ordt[:, w, hh * 832:(hh + 1) * 832],
                            in0=ordt[:, w, hh * 832:(hh + 1) * 832],
                            in1=t[:].rearrange("p h d -> p (h d)"), op=OP.add)
                    return wb_add

                def mk_wb_fin(ordt, j):
                    def wb_fin(w, sb):
                        panel = sb.tile([128, 13, 128], bf16, name="panel")
                        for k in range(13):
                            ptp = ps3c.tile([128, 128], bf16, name="ptp")
                            nc.tensor.transpose(out=ptp[:], in_=ordt[:, w, k * 128:(k + 1) * 128],
                                                identity=ident_t[:])
                            nc.scalar.activation(out=panel[:, k, :], in_=ptp[:], func=AF.Copy)
                        nc.sync.dma_start(
                            out=sT_dram[j][:, :, w * 128:(w + 1) * 128]
                                .rearrange("k p e -> p k e"),
                            in_=panel[:])
                    return wb_fin

                ordt0 = sb3o.tile([128, WCC, D], bf16, name="ordt")
                gat_pass(sb3, ps3a, ps3b, ps3d, tf_tab, SWF, idx_cc_t, dl_cc_t, dlr_cc_d,
                         sd_f, WCC, C_CC, D, blk0=0, writeback=mk_wb_first(ordt0), tag="cc")
                nc.sync.dma_start(out=ord_dram[:].rearrange("(w p) d -> p w d", p=128),
                                  in_=ordt0[:])
                ordt1 = sb3o.tile([128, WCC, D], bf16, name="ordt")
                gat_pass(sb3, ps3a, ps3b, ps3d, tf_tab, SWF, idx_cc_t, dl_cc_t, dlr_cc_d,
                         sd_f, WCC, C_CC, D, blk0=WCC, writeback=mk_wb_first(ordt1), tag="cc")
                gat_pass(sb3, ps3a, ps3b, ps3d, tb_tab, SWF, idx_cc_t, dl_cc_t, dlr_cc_d,
                         sd_b, WCC, C_CC, D, blk0=3 * WCC, writeback=mk_wb_add(ordt1, 1),
                         wb_final=mk_wb_fin(ordt1, 1), tag="cc")
                ordt0b = sb3o.tile([128, WCC, D], bf16, name="ordt")
                nc.sync.dma_start(out=ordt0b[:],
                                  in_=ord_dram[:].rearrange("(w p) d -> p w d", p=128))
                gat_pass(sb3, ps3a, ps3b, ps3d, tb_tab, SWF, idx_cc_t, dl_cc_t, dlr_cc_d,
                         sd_b, WCC, C_CC, D, blk0=2 * WCC, writeback=mk_wb_add(ordt0b, 0),
                         wb_final=mk_wb_fin(ordt0b, 0), tag="cc")

            # -------- P4: attention scores + gate + per-order Wk GEMMs ------
            if PH < 4: break
            with tc.tile_pool(name=f"p4_{it}", bufs=1) as sb4, \
                 tc.tile_pool(name=f"p4s_{it}", bufs=2) as sb4s, \
                 tc.tile_pool(name=f"p4c_{it}", bufs=3) as sb4c, \
                 tc.tile_pool(name=f"p4p_{it}", bufs=2, space="PSUM") as ps4:
                w1_t = sb4.tile([128, 13, 1024], bf16, name="w1t")
                nc.sync.dma_start(out=w1_t[:], in_=w1_d[:].rearrange("(k p) n -> p k n", p=128))
                wk_t = sb4.tile([128, 13, 776], bf16, name="wkt")
                nc.sync.dma_start(out=wk_t[:], in_=wk_ext[:].rearrange("(k p) n -> p k n", p=128))
                scol = [sb4.tile([128, WCC], f32, name=f"scol{j}") for j in range(2)]
                hk_dram = [tk0_dram, tk1_dram]
                for j in (1, 0):
                    for q in range(4):
                        sT = sb4s.tile([128, 13, 512], bf16, name="sTq")
                        nc.sync.dma_start(
                            out=sT[:],
                            in_=sT_dram[j][:, :, q * 512:(q + 1) * 512]
                                .rearrange("k p e -> p k e"))
                        h1 = sb4s.tile([128, 8, 512], bf16, name="h1")
                        for t in range(8):
                            ph = ps4.tile([128, 512], f32, name="ph")
                            for k in range(13):
                                nc.tensor.matmul(out=ph[:], lhsT=w1_t[:, k, t * 128:(t + 1) * 128],
                                                 rhs=sT[:, k, :], start=(k == 0), stop=(k == 12))
                            nc.scalar.activation(out=h1[:, t, :], in_=ph[:], func=AF.Relu,
                                                 bias=b1c_t[:, t:t + 1])
                        if os.environ.get("DBG_P4", "9") < "2": continue
                        for rg in range(4):
                            psc = ps4.tile([128, 16], f32, name="psc")
                            for t in range(8):
                                nc.tensor.matmul(out=psc[:, :1],
                                                 lhsT=h1[:, t, rg * 128:(rg + 1) * 128],
                                                 rhs=w2c_t[:, t:t + 1],
                                                 start=(t == 0), stop=(t == 7))
                            nc.vector.tensor_copy(out=scol[j][:, q * 4 + rg:q * 4 + rg + 1],
                                                  in_=psc[:, :1])
                        if os.environ.get("DBG_P4", "9") < "3": continue
                        for b in range(4):
                            m = q * 4 + b
                            stgh = sb4s.tile([128, 776], bf16, name="stgh")
                            for (n0, nw) in [(0, 512), (512, 264)]:
                                pk = ps4.tile([128, 512], f32, name="pk")
                                for k in range(13):
                                    nc.tensor.matmul(out=pk[:, :nw],
                                                     lhsT=sT[:, k, b * 128:(b + 1) * 128],
                                                     rhs=wk_t[:, k, n0:n0 + nw],
                                                     start=(k == 0), stop=(k == 12))
                                nc.scalar.activation(out=stgh[:, n0:n0 + nw], in_=pk[:, :nw],
                                                     func=AF.Copy)
                            nc.sync.dma_start(out=hk_dram[j][m * 128:(m + 1) * 128, :],
                                              in_=stgh[:])
                # gate w0 = sigmoid(s0 - s1)
                if os.environ.get("DBG_P4", "9") < "4": continue
                wd = sb4.tile([128, WCC], f32, name="wd")
                nc.vector.tensor_tensor(out=wd[:], in0=scol[0][:], in1=scol[1][:], op=OP.subtract)
                w0c = sb4.tile([128, WCC], f32, name="w0c")
                nc.scalar.activation(out=w0c[:], in_=wd[:], func=AF.Sigmoid)
                w1c = sb4.tile([128, WCC], f32, name="w1c")
                nc.vector.tensor_scalar(out=w1c[:], in0=w0c[:], scalar1=-1.0, scalar2=1.0,
                                        op0=OP.mult, op1=OP.add)
                # combine: hk = w0 * hk0 + w1 * hk1, write tk_in, half AllGathers
                if os.environ.get("DBG_P4", "9") < "5": continue
                for m in range(WCC):
                    c0 = sb4c.tile([128, 776], bf16, name="cmb0")
                    nc.sync.dma_start(out=c0[:], in_=tk0_dram[m * 128:(m + 1) * 128, :])
                    c1 = sb4c.tile([128, 776], bf16, name="cmb1")
                    nc.sync.dma_start(out=c1[:], in_=tk1_dram[m * 128:(m + 1) * 128, :])
                    t0 = sb4c.tile([128, 776], bf16, name="t0c")
                    nc.scalar.activation(out=t0[:], in_=c0[:], func=AF.Copy,
                                         scale=w0c[:, m:m + 1])
                    t1 = sb4c.tile([128, 776], bf16, name="t1c")
                    nc.scalar.activation(out=t1[:], in_=c1[:], func=AF.Copy,
                                         scale=w1c[:, m:m + 1])
                    stg = sb4c.tile([128, 776], bf16, name="stgk")
                    nc.vector.tensor_tensor(out=stg[:], in0=t0[:], in1=t1[:], op=OP.add)
                    nc.sync.dma_start(out=tk_in[m * 128:(m + 1) * 128, :776], in_=stg[:])
                    if os.environ.get("DBG_P4", "9") < "6": continue
                    if m == WCC // 2 - 1 and os.environ.get("DBG_AGA", "1") == "1":
                        nc.gpsimd.collective_compute(
                            "AllGather", OP.bypass, replica_groups=RG,
                            ins=[tk_in[0:NCL // 2, :]], outs=[tk_tab[0:NC // 2, :]])
                if os.environ.get("DBG_AGA", "1") == "0":
                    nc.gpsimd.collective_compute(
                        "AllGather", OP.bypass, replica_groups=RG,
                        ins=[tk_in[0:NCL // 2, :]], outs=[tk_tab[0:NC // 2, :]])
                nc.gpsimd.collective_compute(
                    "AllGather", OP.bypass, replica_groups=RG,
                    ins=[tk_in[NCL // 2:NCL, :]], outs=[tk_tab[NC // 2:NC, :]])

            # -------- P5b: sd_q = tok_cons @ Wq_ad --------------------------
            if PH < 5: break
            with tc.tile_pool(name=f"p5b_{it}", bufs=1) as sb5b, \
                 tc.tile_pool(name=f"p5bp_{it}", bufs=2, space="PSUM") as ps5b:
                if tokcT_cur is None:
                    tokcT_cur = kp2.tile([128, 6, NTL], bf16, name="tokcT")
                    nc.sync.dma_start(out=tokcT_cur[:],
                                      in_=tokT[:].rearrange("(k p) e -> p k e", p=128))
                wq_t = sb5b.tile([128, 6, 8], bf16, name="wqt")
                nc.sync.dma_start(out=wq_t[:], in_=wq_ad[:].rearrange("(k p) n -> p k n", p=128))
                sd_q = kp1.tile([128, WCT, 8], bf16, name=f"sd_q{it}")
                for m in range(WCT):
                    pq = ps5b.tile([128, 16], f32, name="pq")
                    for k in range(6):
                        nc.tensor.matmul(out=pq[:, :8], lhsT=tokcT_cur[:, k, m * 128:(m + 1) * 128],
                                         rhs=wq_t[:, k, :], start=(k == 0), stop=(k == 5))
                    nc.scalar.activation(out=sd_q[:, m, :], in_=pq[:, :8], func=AF.Copy)

            # -------- P6: ct GAT -> tok_cons --------------------------------
            if PH < 6: break
            tokcT_next = kp2.tile([128, 6, NTL], bf16, name="tokcT")
            with tc.tile_pool(name=f"p6_{it}", bufs=2) as sb6, \
                 tc.tile_pool(name=f"p6pv_{it}", bufs=2, space="PSUM") as ps6a, \
                 tc.tile_pool(name=f"p6ps_{it}", bufs=2, space="PSUM") as ps6b, \
                 tc.tile_pool(name=f"p6pd_{it}", bufs=1, space="PSUM") as ps6d, \
                 tc.tile_pool(name=f"p6pt_{it}", bufs=2, space="PSUM") as ps6c:

                tkw_tiles = {}

                def wb_tok(w, hh, pv, rec, sb):
                    if hh == 0:
                        tkw_tiles[w] = sb.tile([128, DH], f32, name="tkw")
                    tkw = tkw_tiles[w]
                    nc.vector.tensor_tensor(
                        out=tkw[:, hh * 384:(hh + 1) * 384].rearrange("p (h d) -> p h d", h=4),
                        in0=pv[:, :384].rearrange("p (h d) -> p h d", h=4),
                        in1=rec[:, hh * 4:(hh + 1) * 4].to_broadcast([128, 4, 96]),
                        op=OP.mult)

                def wb_tok_fin(w, sb):
                    tkw = tkw_tiles.pop(w)
                    nc.sync.dma_start(out=tcf_dram[w * 128:(w + 1) * 128, :], in_=tkw[:])
                    stg = sb.tile([128, DH], bf16, name="stgt")
                    nc.vector.tensor_copy(out=stg[:], in_=tkw[:])
                    if it == 0:
                        nc.sync.dma_start(out=tt_in[w * 128:(w + 1) * 128, :], in_=stg[:])
                    for k in range(6):
                        ptp = ps6c.tile([128, 128], bf16, name="ptp6")
                        nc.tensor.transpose(out=ptp[:], in_=stg[:, k * 128:(k + 1) * 128],
                                            identity=ident_t[:])
                        nc.scalar.activation(out=tokcT_next[:, k, w * 128:(w + 1) * 128],
                                             in_=ptp[:], func=AF.Copy)

                gat_pass(sb6, ps6a, ps6b, ps6d, tk_tab, SWK, idx_ct_t, dl_ct_t, dlr_ct_d,
                         sd_q, WCT, C_CT, DH, blk0=0, writeback=wb_tok,
                         wb_final=wb_tok_fin, tag="ct")
                if it == 0:
                    nc.gpsimd.collective_compute("AllGather", OP.bypass, replica_groups=RG,
                                                 ins=[tt_in[:]], outs=[tt_tab[:]])
            tokcT_cur = tokcT_next

        # -------- P7: fuse gate + blend -------------------------------------
        if PH >= 7:
          with tc.tile_pool(name="p7", bufs=1) as sb7, \
              tc.tile_pool(name="p7s", bufs=3) as sb7s, \
              tc.tile_pool(name="p7p", bufs=2, space="PSUM") as ps7:
             fw_t = sb7.tile([128, 12, DH], bf16, name="fwt")
             nc.sync.dma_start(out=fw_t[:], in_=fuse_w_d[:].rearrange("(k p) n -> p k n", p=128))
             fb_t = sb7.tile([1, DH], bf16, name="fbt")
             nc.sync.dma_start(out=fb_t[:], in_=fb_row_d[:])
             tokT_t = sb7.tile([128, 6, NTL], bf16, name="tokTt")
             nc.sync.dma_start(out=tokT_t[:], in_=tokT[:].rearrange("(k p) e -> p k e", p=128))
             for m in range(WCT):
                 f_t = sb7s.tile([128, DH], f32, name="f_t")
                 for (n0, nw) in [(0, 512), (512, 256)]:
                     pf = ps7.tile([128, 512], f32, name="pf")
                     for k in range(12):
                         lt = tokT_t[:, k, m * 128:(m + 1) * 128] if k < 6 else \
                              tokcT_cur[:, k - 6, m * 128:(m + 1) * 128]
                         nc.tensor.matmul(out=pf[:, :nw], lhsT=lt, rhs=fw_t[:, k, n0:n0 + nw],
                                          start=(k == 0), stop=False)
                     nc.tensor.matmul(out=pf[:, :nw], lhsT=ones1_t[:], rhs=fb_t[:, n0:n0 + nw],
                                      start=False, stop=True)
                     nc.scalar.activation(out=f_t[:, n0:n0 + nw], in_=pf[:, :nw], func=AF.Sigmoid)
                 tok_in = sb7s.tile([128, DH], f32, name="tok_in")
                 nc.sync.dma_start(out=tok_in[:], in_=tok_f32_d[m * 128:(m + 1) * 128, :])
                 tcf_in = sb7s.tile([128, DH], f32, name="tcf_in")
                 nc.sync.dma_start(out=tcf_in[:], in_=tcf_dram[m * 128:(m + 1) * 128, :])
                 dlt = sb7s.tile([128, DH], f32, name="dlt")
                 nc.vector.tensor_tensor(out=dlt[:], in0=tok_in[:], in1=tcf_in[:], op=OP.subtract)
                 nc.vector.tensor_tensor(out=dlt[:], in0=f_t[:], in1=dlt[:], op=OP.mult)
                 nc.vector.tensor_tensor(out=dlt[:], in0=tcf_in[:], in1=dlt[:], op=OP.add)
                 nc.sync.dma_start(out=out_d[m * 128:(m + 1) * 128, :], in_=dlt[:])

    nc.finalize()
    return nc


# ------------------------------------------------------------------ driver --

def run(inputs, trace=False):
    in_maps, C_CC, C_CT = _host_prep(inputs)
    nc = _build_nc(C_CC, C_CT)
    res = run_bass_kernel_spmd(nc, in_maps, core_ids=list(range(NCORES)), trace=trace)
    out = np.concatenate([res.results[c]["out"] for c in range(NCORES)], axis=0)
    return out.astype(np.float32), res


def kernel(**inputs) -> np.ndarray:
    out, _ = run(inputs, trace=False)
    return out



# revision 8
# speedup vs baseline: 1.2163x; 1.2163x over previous
"""Trainium2 Bass kernel for nn_CorefModel (GNN message passing, 8 NeuronCores).

Sharding: constituents 2048/core, tokens 1024/core (row shards). Per-iteration
node-feature tables (hf, hb, hk) are produced by per-core GEMMs, AllGathered
into a full per-core HBM copy, and edge gathers (dma_gather) read full rows
from the local copy. GAT edge aggregation runs as one-hot scatter matmuls on
the TensorEngine over destination-sorted edge chunks; the softmax denominator
is accumulated by the same one-hot against exp values.

Perf structure vs v1:
- Feature blocks stored HEAD-MINOR ([dh, H] interleaved) so the per-edge
  alpha broadcast-multiply hits the DVE 2x packed mode (inner step-1 dim).
  Weight matrices are permuted on the host to match (columns for producers,
  rows for consumers); the final output is un-permuted on device.
- One-hot scatter planes (edge->dst and dst->edge) are precomputed on the
  host and streamed from HBM instead of being built with is_equal on DVE.
- cc-GAT passes run (fwd_j, bwd_j) per order j: single live order
  accumulator, no SBUF spill/reload dance.
- Order panels are spilled untransposed and reloaded via xbar DMA transpose
  (replaces hundreds of PE transposes + ACT copies).
- P4 computes attention scores in transposed [1, cons] form (w2 as lhsT),
  gate-combines the two order panels on DVE, then runs a single Wk GEMM.
Compute dtype bf16 with fp32 PSUM accumulation; final blend in fp32.
"""
import sys, os
sys.path.insert(0, '/opt/trn_rl_repo')
import math
import numpy as np
import ml_dtypes

import concourse.bass as bass
from concourse import bacc
import concourse.tile as tile
from concourse import mybir
from concourse.bass_utils import run_bass_kernel_spmd

BF = ml_dtypes.bfloat16
F32 = np.float32

H = 8
NT, NC = 8192, 16384
DH, DCH = 768, 128
D = 2 * DH + DCH          # 1664
NCORES = 8
NCL, NTL = NC // NCORES, NT // NCORES     # 2048, 1024
WCC, WCT = NCL // 128, NTL // 128         # 16, 8
SWF = 1792                 # cc table row stride (bf16 elems, 3584B)
SWK = 896                  # hk table row stride (1792B)

f32 = mybir.dt.float32
bf16 = mybir.dt.bfloat16
i16 = mybir.dt.int16
AF = mybir.ActivationFunctionType
OP = mybir.AluOpType


# ---------------------------------------------------------------- host prep --

def _bdiag(a):
    h, dh = a.shape
    m = np.zeros((h * dh, h), F32)
    for i in range(h):
        m[i * dh:(i + 1) * dh, i] = a[i]
    return m


def _permC(W, h, dh):
    # natural col f = hh*dh+d  ->  head-minor col p = d*h+hh
    r = W.shape[0]
    return np.ascontiguousarray(W.reshape(r, h, dh).transpose(0, 2, 1).reshape(r, h * dh))


def _permR(W, h, dh):
    c = W.shape[1]
    return np.ascontiguousarray(W.reshape(h, dh, c).transpose(1, 0, 2).reshape(h * dh, c))


def _permV(v, h, dh):
    return np.ascontiguousarray(v.reshape(h, dh).T.reshape(h * dh))


def _idx_plane(idxs):
    """[128, n/16] int16: idx i at (i%16, i//16), replicated across 8 Q7 groups."""
    n = len(idxs)
    plane = np.zeros((128, n // 16), np.int16)
    plane[np.arange(n) % 16, np.arange(n) // 16] = idxs
    for g in range(1, 8):
        plane[16 * g:16 * (g + 1)] = plane[:16]
    return plane


def _prep_edges(srcs, dsts, wins):
    """Per edge set: sort by dst, split into per-(core, 128-dst-window) groups.
    Returns uniform chunks/window C and per-core index + one-hot planes:
      oh  [128(e), blocks*C*128] : oh[e, (blk,c,j)] = 1 iff dstloc(edge c*128+e)==j
      ohd [128(j), blocks*C*128] : ohd[j, (blk,c,e)] = 1 iff dstloc(edge c*128+e)==j
    """
    nsets = len(srcs)
    percw = [[[None] * wins for _ in range(nsets)] for _ in range(NCORES)]
    maxe = 0
    for s in range(nsets):
        src, dst = srcs[s], dsts[s]
        gwin = dst // 128
        order = np.argsort(gwin, kind='stable')
        src_s, dst_s, gwin_s = src[order], dst[order], gwin[order]
        bounds = np.searchsorted(gwin_s, np.arange(NCORES * wins + 1))
        for c in range(NCORES):
            for w in range(wins):
                g = c * wins + w
                lo, hi = bounds[g], bounds[g + 1]
                maxe = max(maxe, hi - lo)
                percw[c][s][w] = (src_s[lo:hi], dst_s[lo:hi] % 128)
    C = max(1, math.ceil(maxe / 128))
    E = C * 128
    ar128 = np.arange(128)
    out = []
    for c in range(NCORES):
        nblk = nsets * wins
        idxp = np.zeros((128, nblk * (E // 16)), np.int16)
        oh = np.zeros((128, nblk * E), BF)
        ohd = np.zeros((128, nblk * E), BF)
        for s in range(nsets):
            for w in range(wins):
                src_e, dloc = percw[c][s][w]
                k = len(src_e)
                blk = s * wins + w
                sp = np.zeros(E, np.int16); sp[:k] = src_e.astype(np.int16)
                dp = np.full(E, -1, np.int64); dp[:k] = dloc
                idxp[:, blk * (E // 16):(blk + 1) * (E // 16)] = _idx_plane(sp)
                eq = (dp.reshape(C, 128)[:, :, None] == ar128[None, None, :])  # [c,e,j]
                oh[:, blk * E:(blk + 1) * E] = \
                    eq.transpose(1, 0, 2).reshape(128, E).astype(BF)
                ohd[:, blk * E:(blk + 1) * E] = \
                    eq.transpose(2, 0, 1).reshape(128, E).astype(BF)
        out.append(dict(idx=idxp, oh=oh, ohd=ohd))
    return C, out


def _host_prep(inp):
    tok = np.asarray(inp['token_embeddings'], F32)
    starts = np.asarray(inp['constituent_starts']).astype(np.int64)
    ends = np.asarray(inp['constituent_ends']).astype(np.int64)
    labels = np.asarray(inp['constituent_labels']).astype(np.int64)
    label_emb = np.asarray(inp['cons_type_table'], F32)[labels]        # [NC, 128]

    Wf = np.asarray(inp['Wf'], F32); Wb = np.asarray(inp['Wb'], F32)
    Wk = np.asarray(inp['Wk'], F32); Wq = np.asarray(inp['Wq'], F32)
    # tables: columns head-minor; Wf/Wb rows stay natural (consume cons),
    # Wk rows permuted (consume fused order features).
    Wf_ext = np.concatenate([_permC(Wf, 8, 208),
                             Wf @ _bdiag(np.asarray(inp['af_s'], F32)),
                             Wf @ _bdiag(np.asarray(inp['af_d'], F32))], 1).astype(BF)
    Wb_ext = np.concatenate([_permC(Wb, 8, 208),
                             Wb @ _bdiag(np.asarray(inp['ab_s'], F32)),
                             Wb @ _bdiag(np.asarray(inp['ab_d'], F32))], 1).astype(BF)
    Wk_ext = _permR(np.concatenate([_permC(Wk, 8, 96),
                                    Wk @ _bdiag(np.asarray(inp['act_s'], F32))], 1),
                    8, 208).astype(BF)
    Wq_ad = _permR(Wq @ _bdiag(np.asarray(inp['act_d'], F32)), 8, 96).astype(BF)
    w1 = _permR(np.asarray(inp['attn_w1'], F32), 8, 208).astype(BF)    # [1664, 1024]
    w2c = np.asarray(inp['attn_w2'], F32).reshape(8, 128).T.copy()
    w2r = np.concatenate([np.repeat(w2c, 128, axis=1),
                          np.repeat(-w2c, 128, axis=1)], 1).astype(BF)  # [128, 2*8*128]
    b1c = np.asarray(inp['attn_b1'], F32).reshape(8, 128).T.copy()     # [128, 8] f32
    fw = np.asarray(inp['fuse_w'], F32)
    fuse_w = np.concatenate([fw[:DH], _permR(fw[DH:], 8, 96)], 0)
    fuse_w = _permC(fuse_w, 8, 96).astype(BF)                          # [1536, 768]
    fb_row = _permV(np.asarray(inp['fuse_b'], F32), 8, 96).reshape(1, DH).astype(BF)

    cons = np.concatenate([tok[starts], tok[ends], label_emb], 1)      # [NC, 1664] f32

    cc_src = np.asarray(inp['cc_src']); cc_dst = np.asarray(inp['cc_dst'])
    ct_src = np.asarray(inp['ct_src']); ct_dst = np.asarray(inp['ct_dst'])

    def _remap_half(g):
        # table rows after two half-shard AllGathers: half A = local rows 0:1024
        # of every rank (global rows 0:8192), half B = local rows 1024:2048.
        r = g // NCL
        l = g % NCL
        return np.where(l < NTL, r * NTL + l, NC // 2 + r * NTL + (l - NTL))
    cc_src = _remap_half(cc_src)
    ct_src = _remap_half(ct_src)
    C_CC, cc_meta = _prep_edges([cc_src[s] for s in range(4)], [cc_dst[s] for s in range(4)], WCC)
    C_CT, ct_meta = _prep_edges([ct_src], [ct_dst], WCT)

    ident = np.eye(128, dtype=F32).astype(BF)
    ones1 = np.ones((1, 128), F32).astype(BF)

    in_maps = []
    for c in range(NCORES):
        csl = slice(c * NCL, (c + 1) * NCL)
        tsl = slice(c * NTL, (c + 1) * NTL)
        m = dict(
            consT0=np.ascontiguousarray(cons[csl].T).astype(BF),       # [1664, 2048]
            labelT=np.ascontiguousarray(label_emb[csl].T).astype(BF),  # [128, 2048]
            tokT=np.ascontiguousarray(tok[tsl].T).astype(BF),          # [768, 1024]
            tokTp=np.ascontiguousarray(_permC(tok[tsl], 8, 96).T).astype(BF),
            tok_f32=_permC(tok[tsl], 8, 96),                           # [1024, 768] permuted
            idx_starts=_idx_plane(starts[csl].astype(np.int16)),       # [128, 128]
            idx_ends=_idx_plane(ends[csl].astype(np.int16)),
            wf_ext=Wf_ext, wb_ext=Wb_ext, wk_ext=Wk_ext, wq_ad=Wq_ad,
            w1=w1, w2r=w2r, b1c=b1c, fuse_w=fuse_w, fb_row=fb_row,
            ident=ident, ones1=ones1,
            idx_cc=cc_meta[c]['idx'], oh_cc=cc_meta[c]['oh'], ohd_cc=cc_meta[c]['ohd'],
            idx_ct=ct_meta[c]['idx'], oh_ct=ct_meta[c]['oh'], ohd_ct=ct_meta[c]['ohd'],
        )
        in_maps.append(m)
    return in_maps, C_CC, C_CT


# ------------------------------------------------------------- device build --

def _build_nc(C_CC, C_CT):
    ECC = C_CC * 128
    ECT = C_CT * 128
    nc = bacc.Bacc("TRN2", num_devices=NCORES)

    def ein(name, shape, dt_):
        return nc.dram_tensor(name, shape, dt_, kind="ExternalInput")

    consT0 = ein("consT0", [D, NCL], bf16)
    labelT = ein("labelT", [128, NCL], bf16)
    tokT = ein("tokT", [DH, NTL], bf16)
    tokTp = ein("tokTp", [DH, NTL], bf16)
    tok_f32_d = ein("tok_f32", [NTL, DH], f32)
    idx_starts = ein("idx_starts", [128, NCL // 16], i16)
    idx_ends = ein("idx_ends", [128, NCL // 16], i16)
    wf_ext = ein("wf_ext", [D, 1680], bf16)
    wb_ext = ein("wb_ext", [D, 1680], bf16)
    wk_ext = ein("wk_ext", [D, 776], bf16)
    wq_ad = ein("wq_ad", [DH, 8], bf16)
    w1_d = ein("w1", [D, 1024], bf16)
    w2r_d = ein("w2r", [128, 2048], bf16)
    b1c_d = ein("b1c", [128, 8], f32)
    fuse_w_d = ein("fuse_w", [2 * DH, DH], bf16)
    fb_row_d = ein("fb_row", [1, DH], bf16)
    ident_d = ein("ident", [128, 128], bf16)
    ones1_d = ein("ones1", [1, 128], bf16)
    idx_cc_d = ein("idx_cc", [128, 4 * WCC * (ECC // 16)], i16)
    oh_cc_d = ein("oh_cc", [128, 4 * WCC * ECC], bf16)
    ohd_cc_d = ein("ohd_cc", [128, 4 * WCC * ECC], bf16)
    idx_ct_d = ein("idx_ct", [128, WCT * (ECT // 16)], i16)
    oh_ct_d = ein("oh_ct", [128, WCT * ECT], bf16)
    ohd_ct_d = ein("ohd_ct", [128, WCT * ECT], bf16)

    out_d = nc.dram_tensor("out", [NTL, DH], f32, kind="ExternalOutput")

    tf_in = nc.dram_tensor("tf_in", [NCL, SWF], bf16)
    tb_in = nc.dram_tensor("tb_in", [NCL, SWF], bf16)
    tk_in = nc.dram_tensor("tk_in", [NCL, SWK], bf16)
    tt_in = nc.dram_tensor("tt_in", [NTL, DH], bf16)
    tf_tab = nc.dram_tensor("tf_tab", [NC, SWF], bf16, addr_space="Shared")
    tb_tab = nc.dram_tensor("tb_tab", [NC, SWF], bf16, addr_space="Shared")
    tk_tab = nc.dram_tensor("tk_tab", [NC, SWK], bf16, addr_space="Shared")
    tt_tab = nc.dram_tensor("tt_tab", [NT, DH], bf16, addr_space="Shared")
    ord_dram = [nc.dram_tensor(f"ordp{j}", [NCL, D], bf16) for j in range(2)]
    tkc_dram = nc.dram_tensor("tkc", [NTL, DH], bf16)
    tcf_dram = nc.dram_tensor("tcf", [NTL, DH], f32)

    RG = [list(range(NCORES))]

    with tile.TileContext(nc) as tc:
      with tc.tile_pool(name="const", bufs=1) as cp, \
           tc.tile_pool(name="keep1", bufs=1) as kp1, \
           tc.tile_pool(name="keep2", bufs=2) as kp2:
        def cload(name, dram, shape, dt_):
            t = cp.tile(shape, dt_, name=name)
            nc.sync.dma_start(out=t[:], in_=dram[:])
            return t
        ident_t = cload("ident", ident_d, [128, 128], bf16)
        ones1_t = cload("ones1", ones1_d, [1, 128], bf16)
        idx_cc_t = cload("idx_cc", idx_cc_d, [128, 4 * WCC * (ECC // 16)], i16)
        idx_ct_t = cload("idx_ct", idx_ct_d, [128, WCT * (ECT // 16)], i16)
        w2r_t = cload("w2r", w2r_d, [128, 2048], bf16)
        b1c_t = cload("b1c", b1c_d, [128, 8], f32)

        def gemm_block(ps, lhsT_fn, nkt, w_t, nchunks, stg, tag):
            for (n0, nw) in nchunks:
                pt = ps.tile([128, 512], f32, name=f"gp{tag}")
                for k in range(nkt):
                    nc.tensor.matmul(out=pt[:, :nw], lhsT=lhsT_fn(k),
                                     rhs=w_t[:, k, n0:n0 + nw],
                                     start=(k == 0), stop=(k == nkt - 1))
                nc.scalar.activation(out=stg[:, n0:n0 + nw], in_=pt[:, :nw], func=AF.Copy)

        def gat_pass(sb, ps_pv, ps_sm, tab, stride, idx_t, oh_d, ohd_d, sd_tile,
                     wins, C, dfeat, blk0, writeback, wb_final=lambda w, sb: None, tag=""):
            E = C * 128
            dh = dfeat // 8
            half = dfeat // 2
            hchunks = []
            n0 = 0
            while n0 < half:
                nw = min(512, half - n0)
                hchunks.append((n0, nw)); n0 += nw
            pvsz = 1024 if half > 512 else 512
            for w in range(wins):
                blk = blk0 + w
                g = sb.tile([128, C, stride], bf16, name=f"gw{tag}")
                for c0 in range(0, C, 5):
                    cc = min(5, C - c0)
                    ioff = blk * (E // 16) + c0 * 8
                    if c0 == 0:
                        nc.gpsimd.dma_gather(
                            out_ap=g[:, 0:cc, :], in_ap=tab[:],
                            idxs_ap=idx_t[:, ioff:ioff + cc * 8],
                            num_idxs=cc * 128, num_idxs_reg=cc * 128, elem_size=stride)
                    else:
                        gtmp = sb.tile([128, 5, stride], bf16, name=f"gtmp{tag}")
                        nc.gpsimd.dma_gather(
                            out_ap=gtmp[:, 0:cc, :], in_ap=tab[:],
                            idxs_ap=idx_t[:, ioff:ioff + cc * 8],
                            num_idxs=cc * 128, num_idxs_reg=cc * 128, elem_size=stride)
                        nc.vector.tensor_copy(out=g[:, c0:c0 + cc, :], in_=gtmp[:, 0:cc, :])
                ohw = sb.tile([128, C, 128], bf16, name=f"oh{tag}")
                nc.sync.dma_start(out=ohw[:],
                                  in_=oh_d[:, blk * E:(blk + 1) * E]
                                      .rearrange("p (c j) -> p c j", c=C))
                ohdw = sb.tile([128, C, 128], bf16, name=f"ohd{tag}")
                nc.sync.dma_start(out=ohdw[:],
                                  in_=ohd_d[:, blk * E:(blk + 1) * E]
                                      .rearrange("p (c e) -> p c e", c=C))
                strip = ps_sm.tile([128, 512], f32, name=f"strip{tag}")
                for c in range(C):
                    nc.tensor.matmul(out=strip[:, c * 8:(c + 1) * 8], lhsT=ohdw[:, c, :],
                                     rhs=sd_tile[:, w, :], start=True, stop=False)
                    nc.tensor.matmul(out=strip[:, c * 8:(c + 1) * 8], lhsT=ident_t[:],
                                     rhs=g[:, c, dfeat:dfeat + 8], start=False, stop=True)
                lg = sb.tile([128, C * 8], f32, name=f"lg{tag}")
                nc.scalar.activation(out=lg[:], in_=strip[:, :C * 8], func=AF.Prelu, alpha=0.2)
                ex = sb.tile([128, C, 8], bf16, name=f"ex{tag}")
                nc.scalar.activation(out=ex[:].rearrange("p c e -> p (c e)"), in_=lg[:],
                                     func=AF.Exp)
                rss = []
                for c in range(C):
                    rs = sb.tile([128, dfeat], bf16, name=f"rs{tag}")
                    nc.vector.tensor_tensor(
                        out=rs[:].rearrange("p (d h) -> p d h", h=8),
                        in0=g[:, c, :dfeat].rearrange("p (d h) -> p d h", h=8),
                        in1=ex[:, c, :].unsqueeze(1).broadcast_to([128, dh, 8]),
                        op=OP.mult)
                    rss.append(rs)
                rec = None
                for hh in range(2):
                    pv = ps_pv.tile([128, pvsz], f32, name=f"pv{tag}")
                    for c in range(C):
                        for (n0, nw) in hchunks:
                            nc.tensor.matmul(out=pv[:, n0:n0 + nw], lhsT=ohw[:, c, :],
                                             rhs=rss[c][:, hh * half + n0:hh * half + n0 + nw],
                                             start=(c == 0), stop=(c == C - 1))
                        if hh == 0:
                            nc.tensor.matmul(out=strip[:, 384:392], lhsT=ohw[:, c, :],
                                             rhs=ex[:, c, :], start=(c == 0), stop=(c == C - 1))
                    if hh == 0:
                        dent = sb.tile([128, 8], f32, name=f"dent{tag}")
                        nc.vector.tensor_scalar(out=dent[:], in0=strip[:, 384:392],
                                                scalar1=1e-9, scalar2=None, op0=OP.add)
                        rec = sb.tile([128, 8], f32, name=f"rec{tag}")
                        nc.vector.reciprocal(out=rec[:], in_=dent[:])
                    writeback(w, hh, pv, rec, sb)
                wb_final(w, sb)

        # ============================ iterations ============================
        PH = int(os.environ.get("PHASE_LIMIT", "99"))
        NITER = int(os.environ.get("NITER", "2"))
        for it in range(NITER):
            # -------- P1+P2: consT, hf/hb tables, AllGather -----------------
            with tc.tile_pool(name=f"p2_{it}", bufs=1) as sb2, \
                 tc.tile_pool(name=f"p2s_{it}", bufs=3) as sb2s, \
                 tc.tile_pool(name=f"p2p_{it}", bufs=2, space="PSUM") as ps2:
                consT = sb2.tile([128, 13, NCL], bf16, name="consT")
                if it == 0:
                    nc.sync.dma_start(out=consT[:],
                                      in_=consT0[:].rearrange("(k p) e -> p k e", p=128))
                else:
                    ist = sb2.tile([128, NCL // 16], i16, name="ist")
                    nc.sync.dma_start(out=ist[:], in_=idx_starts[:])
                    ien = sb2.tile([128, NCL // 16], i16, name="ien")
                    nc.sync.dma_start(out=ien[:], in_=idx_ends[:])
                    for half, idxt in ((0, ist), (1, ien)):
                        for q in range(4):
                            gt = sb2s.tile([128, 6, 512], bf16, name="gtc")
                            nc.gpsimd.dma_gather(
                                out_ap=gt[:], in_ap=tt_tab[:],
                                idxs_ap=idxt[:, q * 32:(q + 1) * 32],
                                num_idxs=512, num_idxs_reg=512,
                                elem_size=DH, transpose=True)
                            nc.vector.tensor_copy(
                                out=consT[:, 6 * half:6 * half + 6, q * 512:(q + 1) * 512],
                                in_=gt[:])
                    nc.sync.dma_start(out=consT[:, 12, :], in_=labelT[:])
                sd_f = kp1.tile([128, WCC, 8], bf16, name=f"sd_f{it}")
                sd_b = kp1.tile([128, WCC, 8], bf16, name=f"sd_b{it}")
                for (wd, outd, tabd, sdt) in [(wf_ext, tf_in, tf_tab, sd_f),
                                              (wb_ext, tb_in, tb_tab, sd_b)]:
                    w_t = sb2.tile([128, 13, 1680], bf16, name="wtab")
                    nc.sync.dma_start(out=w_t[:], in_=wd[:].rearrange("(k p) n -> p k n", p=128))
                    for m in range(WCC):
                        stg = sb2s.tile([128, 1680], bf16, name="stg")
                        gemm_block(ps2, lambda k: consT[:, k, m * 128:(m + 1) * 128], 13,
                                   w_t, [(0, 512), (512, 512), (1024, 512), (1536, 144)],
                                   stg, "t")
                        nc.sync.dma_start(out=outd[m * 128:(m + 1) * 128, :1680], in_=stg[:])
                        nc.vector.tensor_copy(out=sdt[:, m, :], in_=stg[:, 1672:1680])
                        if m == WCC // 2 - 1:
                            nc.gpsimd.collective_compute(
                                "AllGather", OP.bypass, replica_groups=RG,
                                ins=[outd[0:NCL // 2, :]], outs=[tabd[0:NC // 2, :]])
                    nc.gpsimd.collective_compute(
                        "AllGather", OP.bypass, replica_groups=RG,
                        ins=[outd[NCL // 2:NCL, :]], outs=[tabd[NC // 2:NC, :]])

            # -------- P3: cc GATs -> order_j, plain spill to ord_dram -------
            if PH < 3: break
            with tc.tile_pool(name=f"p3_{it}", bufs=1) as sb3o, \
                 tc.tile_pool(name=f"p3s_{it}", bufs=3) as sb3, \
                 tc.tile_pool(name=f"p3pv_{it}", bufs=2, space="PSUM") as ps3a, \
                 tc.tile_pool(name=f"p3ps_{it}", bufs=2, space="PSUM") as ps3b:

                def mk_wb_first(ordt):
                    def wb_first(w, hh, pv, rec, sb):
                        nc.vector.tensor_tensor(
                            out=ordt[:, w, hh * 832:(hh + 1) * 832]
                                .rearrange("p (d h) -> p d h", h=8),
                            in0=pv[:, :832].rearrange("p (d h) -> p d h", h=8),
                            in1=rec[:].unsqueeze(1).broadcast_to([128, 104, 8]),
                            op=OP.mult)
                    return wb_first

                def mk_wb_add(ordt):
                    def wb_add(w, hh, pv, rec, sb):
                        t = sb.tile([128, 832], bf16, name="tadd")
                        nc.vector.tensor_tensor(
                            out=t[:].rearrange("p (d h) -> p d h", h=8),
                            in0=pv[:, :832].rearrange("p (d h) -> p d h", h=8),
                            in1=rec[:].unsqueeze(1).broadcast_to([128, 104, 8]),
                            op=OP.mult)
                        nc.vector.tensor_tensor(
                            out=ordt[:, w, hh * 832:(hh + 1) * 832],
                            in0=ordt[:, w, hh * 832:(hh + 1) * 832],
                            in1=t[:], op=OP.add)
                    return wb_add

                def mk_wb_fin(ordt, j):
                    def wb_fin(w, sb):
                        nc.sync.dma_start(out=ord_dram[j][w * 128:(w + 1) * 128, :],
                                          in_=ordt[:, w, :])
                    return wb_fin

                for j in range(2):
                    ordt = sb3o.tile([128, WCC, D], bf16, name="ordt")
                    gat_pass(sb3, ps3a, ps3b, tf_tab, SWF, idx_cc_t, oh_cc_d, ohd_cc_d,
                             sd_f, WCC, C_CC, D, blk0=j * WCC,
                             writeback=mk_wb_first(ordt), tag="cc")
                    gat_pass(sb3, ps3a, ps3b, tb_tab, SWF, idx_cc_t, oh_cc_d, ohd_cc_d,
                             sd_b, WCC, C_CC, D, blk0=(2 + j) * WCC,
                             writeback=mk_wb_add(ordt), wb_final=mk_wb_fin(ordt, j),
                             tag="cc")

            # -------- P4: attention scores + gate + single Wk GEMM ----------
            if PH < 4: break
            with tc.tile_pool(name=f"p4_{it}", bufs=1) as sb4, \
                 tc.tile_pool(name=f"p4s_{it}", bufs=2) as sb4s, \
                 tc.tile_pool(name=f"p4p_{it}", bufs=2, space="PSUM") as ps4, \
                 tc.tile_pool(name=f"p4pr_{it}", bufs=1, space="PSUM") as ps4r:
                w1_t = sb4.tile([128, 13, 1024], bf16, name="w1t")
                nc.sync.dma_start(out=w1_t[:], in_=w1_d[:].rearrange("(k p) n -> p k n", p=128))
                wk_t = sb4.tile([128, 13, 776], bf16, name="wkt")
                nc.sync.dma_start(out=wk_t[:], in_=wk_ext[:].rearrange("(k p) n -> p k n", p=128))
                for q in range(4):
                    sTq = []
                    h1q = []
                    for j in range(2):
                        sT = sb4s.tile([128, 13, 512], bf16, name=f"sT{j}")
                        nc.sync.dma_start_transpose(
                            out=sT[:], in_=ord_dram[j][q * 512:(q + 1) * 512, :])
                        sTq.append(sT)
                        h1 = sb4s.tile([128, 8, 512], bf16, name=f"h1_{j}")
                        for t in range(8):
                            ph = ps4.tile([128, 512], f32, name="ph")
                            for k in range(13):
                                nc.tensor.matmul(out=ph[:], lhsT=w1_t[:, k, t * 128:(t + 1) * 128],
                                                 rhs=sT[:, k, :], start=(k == 0), stop=(k == 12))
                            nc.scalar.activation(out=h1[:, t, :], in_=ph[:], func=AF.Relu,
                                                 bias=b1c_t[:, t:t + 1])
                        h1q.append(h1)
                    psc = ps4r.tile([128, 512], f32, name="psc")
                    for j in range(2):
                        for t in range(8):
                            nc.tensor.matmul(out=psc[:],
                                             lhsT=w2r_t[:, j * 1024 + t * 128:
                                                        j * 1024 + (t + 1) * 128],
                                             rhs=h1q[j][:, t, :],
                                             start=(j == 0 and t == 0),
                                             stop=(j == 1 and t == 7))
                    w0b = sb4s.tile([128, 512], bf16, name="w0b")
                    nc.scalar.activation(out=w0b[:], in_=psc[:], func=AF.Sigmoid)
                    # in-place gate combine: sT0 <- sT1 + w0*(sT0-sT1)
                    sTc = sTq[0]
                    nc.vector.tensor_tensor(out=sTc[:], in0=sTc[:], in1=sTq[1][:],
                                            op=OP.subtract)
                    nc.vector.tensor_tensor(
                        out=sTc[:], in0=sTc[:],
                        in1=w0b[:].unsqueeze(1).broadcast_to([128, 13, 512]),
                        op=OP.mult)
                    nc.vector.tensor_tensor(out=sTc[:], in0=sTc[:], in1=sTq[1][:], op=OP.add)
                    for b in range(4):
                        m = q * 4 + b
                        stgh = sb4s.tile([128, 776], bf16, name="stgh")
                        for (n0, nw) in [(0, 512), (512, 264)]:
                            pk = ps4.tile([128, 512], f32, name="pk")
                            for k in range(13):
                                nc.tensor.matmul(out=pk[:, :nw],
                                                 lhsT=sTc[:, k, b * 128:(b + 1) * 128],
                                                 rhs=wk_t[:, k, n0:n0 + nw],
                                                 start=(k == 0), stop=(k == 12))
                            nc.scalar.activation(out=stgh[:, n0:n0 + nw], in_=pk[:, :nw],
                                                 func=AF.Copy)
                        nc.sync.dma_start(out=tk_in[m * 128:(m + 1) * 128, :776], in_=stgh[:])
                        if m == WCC // 2 - 1:
                            nc.gpsimd.collective_compute(
                                "AllGather", OP.bypass, replica_groups=RG,
                                ins=[tk_in[0:NCL // 2, :]], outs=[tk_tab[0:NC // 2, :]])
                nc.gpsimd.collective_compute(
                    "AllGather", OP.bypass, replica_groups=RG,
                    ins=[tk_in[NCL // 2:NCL, :]], outs=[tk_tab[NC // 2:NC, :]])

            # -------- P5b: sd_q = tok_cons @ Wq_ad --------------------------
            if PH < 5: break
            with tc.tile_pool(name=f"p5b_{it}", bufs=1) as sb5b, \
                 tc.tile_pool(name=f"p5bp_{it}", bufs=2, space="PSUM") as ps5b:
                tokcT = kp2.tile([128, 6, NTL], bf16, name="tokcT")
                if it == 0:
                    nc.sync.dma_start(out=tokcT[:],
                                      in_=tokTp[:].rearrange("(k p) e -> p k e", p=128))
                else:
                    nc.sync.dma_start_transpose(out=tokcT[:], in_=tkc_dram[:])
                wq_t = sb5b.tile([128, 6, 8], bf16, name="wqt")
                nc.sync.dma_start(out=wq_t[:], in_=wq_ad[:].rearrange("(k p) n -> p k n", p=128))
                sd_q = kp1.tile([128, WCT, 8], bf16, name=f"sd_q{it}")
                for m in range(WCT):
                    pq = ps5b.tile([128, 16], f32, name="pq")
                    for k in range(6):
                        nc.tensor.matmul(out=pq[:, :8], lhsT=tokcT[:, k, m * 128:(m + 1) * 128],
                                         rhs=wq_t[:, k, :], start=(k == 0), stop=(k == 5))
                    nc.scalar.activation(out=sd_q[:, m, :], in_=pq[:, :8], func=AF.Copy)

            # -------- P6: ct GAT -> tok_cons --------------------------------
            if PH < 6: break
            with tc.tile_pool(name=f"p6_{it}", bufs=2) as sb6, \
                 tc.tile_pool(name=f"p6pv_{it}", bufs=2, space="PSUM") as ps6a, \
                 tc.tile_pool(name=f"p6ps_{it}", bufs=2, space="PSUM") as ps6b:

                tkw_tiles = {}

                def wb_tok(w, hh, pv, rec, sb):
                    if hh == 0:
                        tkw_tiles[w] = sb.tile([128, DH], f32, name="tkw")
                    tkw = tkw_tiles[w]
                    nc.vector.tensor_tensor(
                        out=tkw[:, hh * 384:(hh + 1) * 384].rearrange("p (d h) -> p d h", h=8),
                        in0=pv[:, :384].rearrange("p (d h) -> p d h", h=8),
                        in1=rec[:].unsqueeze(1).broadcast_to([128, 48, 8]),
                        op=OP.mult)

                def wb_tok_fin(w, sb):
                    tkw = tkw_tiles.pop(w)
                    if it == NITER - 1:
                        nc.sync.dma_start(out=tcf_dram[w * 128:(w + 1) * 128, :], in_=tkw[:])
                    stg = sb.tile([128, DH], bf16, name="stgt")
                    nc.vector.tensor_copy(out=stg[:], in_=tkw[:])
                    nc.sync.dma_start(out=tkc_dram[w * 128:(w + 1) * 128, :], in_=stg[:])
                    if it == 0:
                        stg_nat = sb.tile([128, DH], bf16, name="stgn")
                        nc.vector.tensor_copy(
                            out=stg_nat[:].rearrange("p (h d) -> p h d", h=8),
                            in_=stg[:].rearrange("p (d h) -> p h d", h=8))
                        nc.sync.dma_start(out=tt_in[w * 128:(w + 1) * 128, :], in_=stg_nat[:])

                gat_pass(sb6, ps6a, ps6b, tk_tab, SWK, idx_ct_t, oh_ct_d, ohd_ct_d,
                         sd_q, WCT, C_CT, DH, blk0=0, writeback=wb_tok,
                         wb_final=wb_tok_fin, tag="ct")
                if it == 0:
                    nc.gpsimd.collective_compute("AllGather", OP.bypass, replica_groups=RG,
                                                 ins=[tt_in[:]], outs=[tt_tab[:]])

        # -------- P7: fuse gate + blend -------------------------------------
        if PH >= 7:
          with tc.tile_pool(name="p7", bufs=1) as sb7, \
              tc.tile_pool(name="p7s", bufs=3) as sb7s, \
              tc.tile_pool(name="p7p", bufs=2, space="PSUM") as ps7:
             fw_t = sb7.tile([128, 12, DH], bf16, name="fwt")
             nc.sync.dma_start(out=fw_t[:], in_=fuse_w_d[:].rearrange("(k p) n -> p k n", p=128))
             fb_t = sb7.tile([1, DH], bf16, name="fbt")
             nc.sync.dma_start(out=fb_t[:], in_=fb_row_d[:])
             tokT_t = sb7.tile([128, 6, NTL], bf16, name="tokTt")
             nc.sync.dma_start(out=tokT_t[:], in_=tokT[:].rearrange("(k p) e -> p k e", p=128))
             tokcT_f = sb7.tile([128, 6, NTL], bf16, name="tokcTf")
             nc.sync.dma_start_transpose(out=tokcT_f[:], in_=tkc_dram[:])
             for m in range(WCT):
                 f_t = sb7s.tile([128, DH], f32, name="f_t")
                 for (n0, nw) in [(0, 512), (512, 256)]:
                     pf = ps7.tile([128, 512], f32, name="pf")
                     for k in range(12):
                         lt = tokT_t[:, k, m * 128:(m + 1) * 128] if k < 6 else \
                              tokcT_f[:, k - 6, m * 128:(m + 1) * 128]
                         nc.tensor.matmul(out=pf[:, :nw], lhsT=lt, rhs=fw_t[:, k, n0:n0 + nw],
                                          start=(k == 0), stop=False)
                     nc.tensor.matmul(out=pf[:, :nw], lhsT=ones1_t[:], rhs=fb_t[:, n0:n0 + nw],
                                      start=False, stop=True)
                     nc.scalar.activation(out=f_t[:, n0:n0 + nw], in_=pf[:, :nw], func=AF.Sigmoid)
                 tok_in = sb7s.tile([128, DH], f32, name="tok_in")
                 nc.sync.dma_start(out=tok_in[:], in_=tok_f32_d[m * 128:(m + 1) * 128, :])
                 tcf_in = sb7s.tile([128, DH], f32, name="tcf_in")
                 nc.sync.dma_start(out=tcf_in[:], in_=tcf_dram[m * 128:(m + 1) * 128, :])
                 dlt = sb7s.tile([128, DH], f32, name="dlt")
                 nc.vector.tensor_tensor(out=dlt[:], in0=tok_in[:], in1=tcf_in[:], op=OP.subtract)
                 nc.vector.tensor_tensor(out=dlt[:], in0=f_t[:], in1=dlt[:], op=OP.mult)
                 nc.vector.tensor_tensor(out=dlt[:], in0=tcf_in[:], in1=dlt[:], op=OP.add)
                 udlt = sb7s.tile([128, DH], f32, name="udlt")
                 nc.vector.tensor_copy(
                     out=udlt[:].rearrange("p (h d) -> p h d", h=8),
                     in_=dlt[:].rearrange("p (d h) -> p h d", h=8))
                 nc.sync.dma_start(out=out_d[m * 128:(m + 1) * 128, :], in_=udlt[:])

    nc.finalize()
    return nc


# ------------------------------------------------------------------ driver --

def run(inputs, trace=False):
    in_maps, C_CC, C_CT = _host_prep(inputs)
    nc = _build_nc(C_CC, C_CT)
    res = run_bass_kernel_spmd(nc, in_maps, core_ids=list(range(NCORES)), trace=trace)
    out = np.concatenate([res.results[c]["out"] for c in range(NCORES)], axis=0)
    return out.astype(np.float32), res


def kernel(**inputs) -> np.ndarray:
    out, _ = run(inputs, trace=False)
    return out


# revision 12
# speedup vs baseline: 1.2467x; 1.0250x over previous
"""Trainium2 Bass kernel for nn_CorefModel (GNN message passing, 8 NeuronCores).

Sharding: constituents 2048/core, tokens 1024/core (row shards). Per-iteration
node-feature tables (hf, hb, hk) are produced by per-core GEMMs, AllGathered
into a full per-core HBM copy, and edge gathers (dma_gather) read full rows
from the local copy. GAT edge aggregation runs as one-hot scatter matmuls on
the TensorEngine over destination-sorted edge chunks; the softmax denominator
is accumulated by the same one-hot against exp values.

Perf structure vs v1:
- Feature blocks stored HEAD-MINOR ([dh, H] interleaved) so the per-edge
  alpha broadcast-multiply hits the DVE 2x packed mode (inner step-1 dim).
  Weight matrices are permuted on the host to match (columns for producers,
  rows for consumers); the final output is un-permuted on device.
- One-hot scatter planes (edge->dst and dst->edge) are precomputed on the
  host and streamed from HBM instead of being built with is_equal on DVE.
- cc-GAT passes run (fwd_j, bwd_j) per order j: single live order
  accumulator, no SBUF spill/reload dance.
- Order panels are spilled untransposed and reloaded via xbar DMA transpose
  (replaces hundreds of PE transposes + ACT copies).
- P4 computes attention scores in transposed [1, cons] form (w2 as lhsT),
  gate-combines the two order panels on DVE, then runs a single Wk GEMM.
Compute dtype bf16 with fp32 PSUM accumulation; final blend in fp32.
"""
import sys, os
sys.path.insert(0, '/opt/trn_rl_repo')
import math
import numpy as np
import ml_dtypes

import concourse.bass as bass
from concourse import bacc
import concourse.tile as tile
from concourse import mybir
from concourse.bass_utils import run_bass_kernel_spmd

BF = ml_dtypes.bfloat16
F32 = np.float32

H = 8
NT, NC = 8192, 16384
DH, DCH = 768, 128
D = 2 * DH + DCH          # 1664
NCORES = 8
NCL, NTL = NC // NCORES, NT // NCORES     # 2048, 1024
WCC, WCT = NCL // 128, NTL // 128         # 16, 8
SWF = 1792                 # cc table row stride (bf16 elems, 3584B)
SWK = 896                  # hk table row stride (1792B)

f32 = mybir.dt.float32
bf16 = mybir.dt.bfloat16
i16 = mybir.dt.int16
AF = mybir.ActivationFunctionType
OP = mybir.AluOpType


# ---------------------------------------------------------------- host prep --

def _bdiag(a):
    h, dh = a.shape
    m = np.zeros((h * dh, h), F32)
    for i in range(h):
        m[i * dh:(i + 1) * dh, i] = a[i]
    return m


def _permC(W, h, dh):
    # natural col f = hh*dh+d  ->  head-minor col p = d*h+hh
    r = W.shape[0]
    return np.ascontiguousarray(W.reshape(r, h, dh).transpose(0, 2, 1).reshape(r, h * dh))


def _permR(W, h, dh):
    c = W.shape[1]
    return np.ascontiguousarray(W.reshape(h, dh, c).transpose(1, 0, 2).reshape(h * dh, c))


def _permV(v, h, dh):
    return np.ascontiguousarray(v.reshape(h, dh).T.reshape(h * dh))


def _idx_plane(idxs):
    """[128, n/16] int16: idx i at (i%16, i//16), replicated across 8 Q7 groups."""
    n = len(idxs)
    plane = np.zeros((128, n // 16), np.int16)
    plane[np.arange(n) % 16, np.arange(n) // 16] = idxs
    for g in range(1, 8):
        plane[16 * g:16 * (g + 1)] = plane[:16]
    return plane


def _prep_edges(srcs, dsts, wins):
    """Per edge set: sort by dst, split into per-(core, 128-dst-window) groups.
    Returns uniform chunks/window C and per-core index + one-hot planes:
      oh  [128(e), blocks*C*128] : oh[e, (blk,c,j)] = 1 iff dstloc(edge c*128+e)==j
      ohd [128(j), blocks*C*128] : ohd[j, (blk,c,e)] = 1 iff dstloc(edge c*128+e)==j
    """
    nsets = len(srcs)
    percw = [[[None] * wins for _ in range(nsets)] for _ in range(NCORES)]
    maxe = 0
    for s in range(nsets):
        src, dst = srcs[s], dsts[s]
        gwin = dst // 128
        order = np.argsort(gwin, kind='stable')
        src_s, dst_s, gwin_s = src[order], dst[order], gwin[order]
        bounds = np.searchsorted(gwin_s, np.arange(NCORES * wins + 1))
        for c in range(NCORES):
            for w in range(wins):
                g = c * wins + w
                lo, hi = bounds[g], bounds[g + 1]
                maxe = max(maxe, hi - lo)
                percw[c][s][w] = (src_s[lo:hi], dst_s[lo:hi] % 128)
    C = max(1, math.ceil(maxe / 128))
    E = C * 128
    ar128 = np.arange(128)
    out = []
    for c in range(NCORES):
        nblk = nsets * wins
        idxp = np.zeros((128, nblk * (E // 16)), np.int16)
        oh = np.zeros((128, nblk * E), BF)
        ohd = np.zeros((128, nblk * E), BF)
        for s in range(nsets):
            for w in range(wins):
                src_e, dloc = percw[c][s][w]
                k = len(src_e)
                blk = s * wins + w
                sp = np.zeros(E, np.int16); sp[:k] = src_e.astype(np.int16)
                dp = np.full(E, -1, np.int64); dp[:k] = dloc
                idxp[:, blk * (E // 16):(blk + 1) * (E // 16)] = _idx_plane(sp)
                eq = (dp.reshape(C, 128)[:, :, None] == ar128[None, None, :])  # [c,e,j]
                oh[:, blk * E:(blk + 1) * E] = \
                    eq.transpose(1, 0, 2).reshape(128, E).astype(BF)
                ohd[:, blk * E:(blk + 1) * E] = \
                    eq.transpose(2, 0, 1).reshape(128, E).astype(BF)
        out.append(dict(idx=idxp, oh=oh, ohd=ohd))
    return C, out


def _host_prep(inp):
    tok = np.asarray(inp['token_embeddings'], F32)
    starts = np.asarray(inp['constituent_starts']).astype(np.int64)
    ends = np.asarray(inp['constituent_ends']).astype(np.int64)
    labels = np.asarray(inp['constituent_labels']).astype(np.int64)
    label_emb = np.asarray(inp['cons_type_table'], F32)[labels]        # [NC, 128]

    Wf = np.asarray(inp['Wf'], F32); Wb = np.asarray(inp['Wb'], F32)
    Wk = np.asarray(inp['Wk'], F32); Wq = np.asarray(inp['Wq'], F32)
    # tables: columns head-minor; Wf/Wb rows stay natural (consume cons),
    # Wk rows permuted (consume fused order features).
    Wf_ext = np.concatenate([_permC(Wf, 8, 208),
                             Wf @ _bdiag(np.asarray(inp['af_s'], F32)),
                             Wf @ _bdiag(np.asarray(inp['af_d'], F32))], 1).astype(BF)
    Wb_ext = np.concatenate([_permC(Wb, 8, 208),
                             Wb @ _bdiag(np.asarray(inp['ab_s'], F32)),
                             Wb @ _bdiag(np.asarray(inp['ab_d'], F32))], 1).astype(BF)
    Wk_ext = _permR(np.concatenate([_permC(Wk, 8, 96),
                                    Wk @ _bdiag(np.asarray(inp['act_s'], F32))], 1),
                    8, 208).astype(BF)
    Wq_ad = _permR(Wq @ _bdiag(np.asarray(inp['act_d'], F32)), 8, 96).astype(BF)
    w1 = _permR(np.asarray(inp['attn_w1'], F32), 8, 208).astype(BF)    # [1664, 1024]
    w2c = np.asarray(inp['attn_w2'], F32).reshape(8, 128).T.copy()
    w2r = np.concatenate([np.repeat(w2c, 128, axis=1),
                          np.repeat(-w2c, 128, axis=1)], 1).astype(BF)  # [128, 2*8*128]
    b1c = np.asarray(inp['attn_b1'], F32).reshape(8, 128).T.copy()     # [128, 8] f32
    fw = np.asarray(inp['fuse_w'], F32)
    fuse_w = np.concatenate([fw[:DH], _permR(fw[DH:], 8, 96)], 0)
    fuse_w = _permC(fuse_w, 8, 96).astype(BF)                          # [1536, 768]
    fb_row = _permV(np.asarray(inp['fuse_b'], F32), 8, 96).reshape(1, DH).astype(BF)

    cons = np.concatenate([tok[starts], tok[ends], label_emb], 1)      # [NC, 1664] f32

    cc_src = np.asarray(inp['cc_src']); cc_dst = np.asarray(inp['cc_dst'])
    ct_src = np.asarray(inp['ct_src']); ct_dst = np.asarray(inp['ct_dst'])

    def _remap512(g, local):
        # table rows after 512-row piecewise AllGathers: piece p holds local
        # rows p*512:(p+1)*512 of every rank at tab[p*4096 + r*512 + (l%512)].
        r = g // local
        l = g % local
        return (l // 512) * (NCORES * 512) + r * 512 + (l % 512)
    cc_src = _remap512(cc_src, NCL)
    ct_src = _remap512(ct_src, NCL)
    starts_t = _remap512(starts, NTL)
    ends_t = _remap512(ends, NTL)
    C_CC, cc_meta = _prep_edges([cc_src[s] for s in range(4)], [cc_dst[s] for s in range(4)], WCC)
    C_CT, ct_meta = _prep_edges([ct_src], [ct_dst], WCT)

    ident = np.eye(128, dtype=F32).astype(BF)
    ones1 = np.ones((1, 128), F32).astype(BF)

    in_maps = []
    for c in range(NCORES):
        csl = slice(c * NCL, (c + 1) * NCL)
        tsl = slice(c * NTL, (c + 1) * NTL)
        m = dict(
            consT0=np.ascontiguousarray(cons[csl].T).astype(BF),       # [1664, 2048]
            labelT=np.ascontiguousarray(label_emb[csl].T).astype(BF),  # [128, 2048]
            tokT=np.ascontiguousarray(tok[tsl].T).astype(BF),          # [768, 1024]
            tokTp=np.ascontiguousarray(_permC(tok[tsl], 8, 96).T).astype(BF),
            tok_f32=_permC(tok[tsl], 8, 96),                           # [1024, 768] permuted
            idx_starts=_idx_plane(starts_t[csl].astype(np.int16)),     # [128, 128]
            idx_ends=_idx_plane(ends_t[csl].astype(np.int16)),
            wf_ext=Wf_ext, wb_ext=Wb_ext, wk_ext=Wk_ext, wq_ad=Wq_ad,
            w1=w1, w2r=w2r, b1c=b1c, fuse_w=fuse_w, fb_row=fb_row,
            ident=ident, ones1=ones1,
            idx_cc=cc_meta[c]['idx'], oh_cc=cc_meta[c]['oh'], ohd_cc=cc_meta[c]['ohd'],
            idx_ct=ct_meta[c]['idx'], oh_ct=ct_meta[c]['oh'], ohd_ct=ct_meta[c]['ohd'],
        )
        in_maps.append(m)
    return in_maps, C_CC, C_CT


# ------------------------------------------------------------- device build --

def _build_nc(C_CC, C_CT):
    ECC = C_CC * 128
    ECT = C_CT * 128
    nc = bacc.Bacc("TRN2", num_devices=NCORES)

    def ein(name, shape, dt_):
        return nc.dram_tensor(name, shape, dt_, kind="ExternalInput")

    consT0 = ein("consT0", [D, NCL], bf16)
    labelT = ein("labelT", [128, NCL], bf16)
    tokT = ein("tokT", [DH, NTL], bf16)
    tokTp = ein("tokTp", [DH, NTL], bf16)
    tok_f32_d = ein("tok_f32", [NTL, DH], f32)
    idx_starts = ein("idx_starts", [128, NCL // 16], i16)
    idx_ends = ein("idx_ends", [128, NCL // 16], i16)
    wf_ext = ein("wf_ext", [D, 1680], bf16)
    wb_ext = ein("wb_ext", [D, 1680], bf16)
    wk_ext = ein("wk_ext", [D, 776], bf16)
    wq_ad = ein("wq_ad", [DH, 8], bf16)
    w1_d = ein("w1", [D, 1024], bf16)
    w2r_d = ein("w2r", [128, 2048], bf16)
    b1c_d = ein("b1c", [128, 8], f32)
    fuse_w_d = ein("fuse_w", [2 * DH, DH], bf16)
    fb_row_d = ein("fb_row", [1, DH], bf16)
    ident_d = ein("ident", [128, 128], bf16)
    ones1_d = ein("ones1", [1, 128], bf16)
    idx_cc_d = ein("idx_cc", [128, 4 * WCC * (ECC // 16)], i16)
    oh_cc_d = ein("oh_cc", [128, 4 * WCC * ECC], bf16)
    ohd_cc_d = ein("ohd_cc", [128, 4 * WCC * ECC], bf16)
    idx_ct_d = ein("idx_ct", [128, WCT * (ECT // 16)], i16)
    oh_ct_d = ein("oh_ct", [128, WCT * ECT], bf16)
    ohd_ct_d = ein("ohd_ct", [128, WCT * ECT], bf16)

    out_d = nc.dram_tensor("out", [NTL, DH], f32, kind="ExternalOutput")

    tf_in = nc.dram_tensor("tf_in", [NCL, SWF], bf16)
    tb_in = nc.dram_tensor("tb_in", [NCL, SWF], bf16)
    tk_in = nc.dram_tensor("tk_in", [NCL, SWK], bf16)
    tt_in = nc.dram_tensor("tt_in", [NTL, DH], bf16)
    tf_tab = nc.dram_tensor("tf_tab", [NC, SWF], bf16, addr_space="Shared")
    tb_tab = nc.dram_tensor("tb_tab", [NC, SWF], bf16, addr_space="Shared")
    tk_tab = nc.dram_tensor("tk_tab", [NC, SWK], bf16, addr_space="Shared")
    tt_tab = nc.dram_tensor("tt_tab", [NT, DH], bf16, addr_space="Shared")
    ord_dram = [nc.dram_tensor(f"ordp{j}", [NCL, D], bf16) for j in range(2)]
    tkc_dram = nc.dram_tensor("tkc", [NTL, DH], bf16)
    tcf_dram = nc.dram_tensor("tcf", [NTL, DH], f32)

    RG = [list(range(NCORES))]

    with tile.TileContext(nc) as tc:
      with tc.tile_pool(name="const", bufs=1) as cp, \
           tc.tile_pool(name="keep1", bufs=1) as kp1, \
           tc.tile_pool(name="keep2", bufs=2) as kp2:
        def cload(name, dram, shape, dt_):
            t = cp.tile(shape, dt_, name=name)
            nc.sync.dma_start(out=t[:], in_=dram[:])
            return t
        ident_t = cload("ident", ident_d, [128, 128], bf16)
        ones1_t = cload("ones1", ones1_d, [1, 128], bf16)
        idx_cc_t = cload("idx_cc", idx_cc_d, [128, 4 * WCC * (ECC // 16)], i16)
        idx_ct_t = cload("idx_ct", idx_ct_d, [128, WCT * (ECT // 16)], i16)
        w2r_t = cload("w2r", w2r_d, [128, 2048], bf16)
        b1c_t = cload("b1c", b1c_d, [128, 8], f32)

        def gemm_block(ps, lhsT_fn, nkt, w_t, nchunks, stg, tag):
            for (n0, nw) in nchunks:
                pt = ps.tile([128, 512], f32, name=f"gp{tag}")
                for k in range(nkt):
                    nc.tensor.matmul(out=pt[:, :nw], lhsT=lhsT_fn(k),
                                     rhs=w_t[:, k, n0:n0 + nw],
                                     start=(k == 0), stop=(k == nkt - 1))
                nc.scalar.activation(out=stg[:, n0:n0 + nw], in_=pt[:, :nw], func=AF.Copy)

        def gat_pass(sb, ps_pv, ps_sm, tab, stride, idx_t, oh_d, ohd_d, sd_tile,
                     wins, C, dfeat, blk0, writeback, wb_final=lambda w, sb: None, tag=""):
            E = C * 128
            dh = dfeat // 8
            half = dfeat // 2
            hchunks = []
            n0 = 0
            while n0 < half:
                nw = min(512, half - n0)
                hchunks.append((n0, nw)); n0 += nw
            pvsz = 1024 if half > 512 else 512
            gstep = 5
            for w in range(wins):
                blk = blk0 + w
                segs = []
                for c0 in range(0, C, gstep):
                    cc = min(gstep, C - c0)
                    ioff = blk * (E // 16) + c0 * 8
                    gt = sb.tile([128, cc, stride], bf16, name=f"gw{tag}{c0}")
                    nc.gpsimd.dma_gather(
                        out_ap=gt[:, 0:cc, :], in_ap=tab[:],
                        idxs_ap=idx_t[:, ioff:ioff + cc * 8],
                        num_idxs=cc * 128, num_idxs_reg=cc * 128, elem_size=stride)
                    segs.append((c0, cc, gt))

                def gch(c):
                    for (c0, cc, gt) in segs:
                        if c0 <= c < c0 + cc:
                            return gt[:, c - c0, :]
                ohw = sb.tile([128, C, 128], bf16, name=f"oh{tag}")
                nc.sync.dma_start(out=ohw[:],
                                  in_=oh_d[:, blk * E:(blk + 1) * E]
                                      .rearrange("p (c j) -> p c j", c=C))
                ohdw = sb.tile([128, C, 128], bf16, name=f"ohd{tag}")
                nc.sync.dma_start(out=ohdw[:],
                                  in_=ohd_d[:, blk * E:(blk + 1) * E]
                                      .rearrange("p (c e) -> p c e", c=C))
                strip = ps_sm.tile([128, 512], f32, name=f"strip{tag}")
                for c in range(C):
                    nc.tensor.matmul(out=strip[:, c * 8:(c + 1) * 8], lhsT=ohdw[:, c, :],
                                     rhs=sd_tile[:, w, :], start=True, stop=False)
                    nc.tensor.matmul(out=strip[:, c * 8:(c + 1) * 8], lhsT=ident_t[:],
                                     rhs=gch(c)[:, dfeat:dfeat + 8], start=False, stop=True)
                lg = sb.tile([128, C * 8], f32, name=f"lg{tag}")
                nc.scalar.activation(out=lg[:], in_=strip[:, :C * 8], func=AF.Prelu, alpha=0.2)
                ex = sb.tile([128, C, 8], bf16, name=f"ex{tag}")
                nc.scalar.activation(out=ex[:].rearrange("p c e -> p (c e)"), in_=lg[:],
                                     func=AF.Exp)
                rss = []
                for c in range(C):
                    rs = sb.tile([128, dfeat], bf16, name=f"rs{tag}")
                    nc.vector.tensor_tensor(
                        out=rs[:].rearrange("p (d h) -> p d h", h=8),
                        in0=gch(c)[:, :dfeat].rearrange("p (d h) -> p d h", h=8),
                        in1=ex[:, c, :].unsqueeze(1).broadcast_to([128, dh, 8]),
                        op=OP.mult)
                    rss.append(rs)
                rec = None
                for hh in range(2):
                    pv = ps_pv.tile([128, pvsz], f32, name=f"pv{tag}")
                    for c in range(C):
                        for (n0, nw) in hchunks:
                            nc.tensor.matmul(out=pv[:, n0:n0 + nw], lhsT=ohw[:, c, :],
                                             rhs=rss[c][:, hh * half + n0:hh * half + n0 + nw],
                                             start=(c == 0), stop=(c == C - 1))
                        if hh == 0:
                            nc.tensor.matmul(out=strip[:, 384:392], lhsT=ohw[:, c, :],
                                             rhs=ex[:, c, :], start=(c == 0), stop=(c == C - 1))
                    if hh == 0:
                        dent = sb.tile([128, 8], f32, name=f"dent{tag}")
                        nc.vector.tensor_scalar(out=dent[:], in0=strip[:, 384:392],
                                                scalar1=1e-9, scalar2=None, op0=OP.add)
                        rec = sb.tile([128, 8], f32, name=f"rec{tag}")
                        nc.vector.reciprocal(out=rec[:], in_=dent[:])
                    writeback(w, hh, pv, rec, sb)
                wb_final(w, sb)

        # ============================ iterations ============================
        PH = int(os.environ.get("PHASE_LIMIT", "99"))
        NITER = int(os.environ.get("NITER", "2"))
        for it in range(NITER):
            # -------- P1+P2: consT, hf/hb tables, AllGather -----------------
            with tc.tile_pool(name=f"p2_{it}", bufs=1) as sb2, \
                 tc.tile_pool(name=f"p2s_{it}", bufs=3) as sb2s, \
                 tc.tile_pool(name=f"p2p_{it}", bufs=2, space="PSUM") as ps2:
                consT = sb2.tile([128, 13, NCL], bf16, name="consT")
                if it == 0:
                    nc.sync.dma_start(out=consT[:],
                                      in_=consT0[:].rearrange("(k p) e -> p k e", p=128))
                else:
                    ist = sb2.tile([128, NCL // 16], i16, name="ist")
                    nc.sync.dma_start(out=ist[:], in_=idx_starts[:])
                    ien = sb2.tile([128, NCL // 16], i16, name="ien")
                    nc.sync.dma_start(out=ien[:], in_=idx_ends[:])
                    for half, idxt in ((0, ist), (1, ien)):
                        for q in range(4):
                            gt = sb2s.tile([128, 6, 512], bf16, name="gtc")
                            nc.gpsimd.dma_gather(
                                out_ap=gt[:], in_ap=tt_tab[:],
                                idxs_ap=idxt[:, q * 32:(q + 1) * 32],
                                num_idxs=512, num_idxs_reg=512,
                                elem_size=DH, transpose=True)
                            nc.vector.tensor_copy(
                                out=consT[:, 6 * half:6 * half + 6, q * 512:(q + 1) * 512],
                                in_=gt[:])
                    nc.sync.dma_start(out=consT[:, 12, :], in_=labelT[:])
                sd_f = kp1.tile([128, WCC, 8], bf16, name=f"sd_f{it}")
                sd_b = kp1.tile([128, WCC, 8], bf16, name=f"sd_b{it}")
                for (wd, outd, tabd, sdt) in [(wf_ext, tf_in, tf_tab, sd_f),
                                              (wb_ext, tb_in, tb_tab, sd_b)]:
                    w_t = sb2.tile([128, 13, 1680], bf16, name="wtab")
                    nc.sync.dma_start(out=w_t[:], in_=wd[:].rearrange("(k p) n -> p k n", p=128))
                    for m in range(WCC):
                        stg = sb2s.tile([128, 1680], bf16, name="stg")
                        gemm_block(ps2, lambda k: consT[:, k, m * 128:(m + 1) * 128], 13,
                                   w_t, [(0, 512), (512, 512), (1024, 512), (1536, 144)],
                                   stg, "t")
                        nc.sync.dma_start(out=outd[m * 128:(m + 1) * 128, :1680], in_=stg[:])
                        nc.vector.tensor_copy(out=sdt[:, m, :], in_=stg[:, 1672:1680])
                        if m % 4 == 3:
                            q = m // 4
                            nc.gpsimd.collective_compute(
                                "AllGather", OP.bypass, replica_groups=RG,
                                ins=[outd[q * 512:(q + 1) * 512, :]],
                                outs=[tabd[q * 4096:(q + 1) * 4096, :]])

            # -------- P3: cc GATs -> order_j, plain spill to ord_dram -------
            if PH < 3: break
            with tc.tile_pool(name=f"p3_{it}", bufs=1) as sb3o, \
                 tc.tile_pool(name=f"p3s_{it}", bufs=4) as sb3, \
                 tc.tile_pool(name=f"p3pv_{it}", bufs=2, space="PSUM") as ps3a, \
                 tc.tile_pool(name=f"p3ps_{it}", bufs=2, space="PSUM") as ps3b:

                def mk_wb_first(ordt):
                    def wb_first(w, hh, pv, rec, sb):
                        nc.vector.tensor_tensor(
                            out=ordt[:, w, hh * 832:(hh + 1) * 832]
                                .rearrange("p (d h) -> p d h", h=8),
                            in0=pv[:, :832].rearrange("p (d h) -> p d h", h=8),
                            in1=rec[:].unsqueeze(1).broadcast_to([128, 104, 8]),
                            op=OP.mult)
                    return wb_first

                def mk_wb_add(ordt):
                    def wb_add(w, hh, pv, rec, sb):
                        t = sb.tile([128, 832], bf16, name="tadd")
                        nc.vector.tensor_tensor(
                            out=t[:].rearrange("p (d h) -> p d h", h=8),
                            in0=pv[:, :832].rearrange("p (d h) -> p d h", h=8),
                            in1=rec[:].unsqueeze(1).broadcast_to([128, 104, 8]),
                            op=OP.mult)
                        nc.vector.tensor_tensor(
                            out=ordt[:, w, hh * 832:(hh + 1) * 832],
                            in0=ordt[:, w, hh * 832:(hh + 1) * 832],
                            in1=t[:], op=OP.add)
                    return wb_add

                def mk_wb_fin(ordt, j):
                    def wb_fin(w, sb):
                        nc.sync.dma_start(out=ord_dram[j][w * 128:(w + 1) * 128, :],
                                          in_=ordt[:, w, :])
                    return wb_fin

                ordt0 = sb3o.tile([128, WCC, D], bf16, name="ordt")
                gat_pass(sb3, ps3a, ps3b, tf_tab, SWF, idx_cc_t, oh_cc_d, ohd_cc_d,
                         sd_f, WCC, C_CC, D, blk0=0,
                         writeback=mk_wb_first(ordt0), tag="cc")
                nc.sync.dma_start(out=ord_dram[0][:].rearrange("(w p) d -> p w d", p=128),
                                  in_=ordt0[:])
                ordt1 = sb3o.tile([128, WCC, D], bf16, name="ordt")
                gat_pass(sb3, ps3a, ps3b, tf_tab, SWF, idx_cc_t, oh_cc_d, ohd_cc_d,
                         sd_f, WCC, C_CC, D, blk0=WCC,
                         writeback=mk_wb_first(ordt1), tag="cc")
                gat_pass(sb3, ps3a, ps3b, tb_tab, SWF, idx_cc_t, oh_cc_d, ohd_cc_d,
                         sd_b, WCC, C_CC, D, blk0=3 * WCC,
                         writeback=mk_wb_add(ordt1), wb_final=mk_wb_fin(ordt1, 1),
                         tag="cc")
                ordt0b = sb3o.tile([128, WCC, D], bf16, name="ordt")
                nc.sync.dma_start(out=ordt0b[:],
                                  in_=ord_dram[0][:].rearrange("(w p) d -> p w d", p=128))
                gat_pass(sb3, ps3a, ps3b, tb_tab, SWF, idx_cc_t, oh_cc_d, ohd_cc_d,
                         sd_b, WCC, C_CC, D, blk0=2 * WCC,
                         writeback=mk_wb_add(ordt0b), wb_final=mk_wb_fin(ordt0b, 0),
                         tag="cc")

            # -------- P4: attention scores + gate + single Wk GEMM ----------
            if PH < 4: break
            with tc.tile_pool(name=f"p4_{it}", bufs=1) as sb4, \
                 tc.tile_pool(name=f"p4s_{it}", bufs=2) as sb4s, \
                 tc.tile_pool(name=f"p4p_{it}", bufs=2, space="PSUM") as ps4, \
                 tc.tile_pool(name=f"p4pr_{it}", bufs=1, space="PSUM") as ps4r:
                w1_t = sb4.tile([128, 13, 1024], bf16, name="w1t")
                nc.sync.dma_start(out=w1_t[:], in_=w1_d[:].rearrange("(k p) n -> p k n", p=128))
                wk_t = sb4.tile([128, 13, 776], bf16, name="wkt")
                nc.sync.dma_start(out=wk_t[:], in_=wk_ext[:].rearrange("(k p) n -> p k n", p=128))
                for q in range(4):
                    sTq = []
                    h1q = []
                    for j in range(2):
                        sT = sb4s.tile([128, 13, 512], bf16, name=f"sT{j}")
                        nc.sync.dma_start_transpose(
                            out=sT[:], in_=ord_dram[j][q * 512:(q + 1) * 512, :])
                        sTq.append(sT)
                        h1 = sb4s.tile([128, 8, 512], bf16, name=f"h1_{j}")
                        for t in range(8):
                            ph = ps4.tile([128, 512], f32, name="ph")
                            for k in range(13):
                                nc.tensor.matmul(out=ph[:], lhsT=w1_t[:, k, t * 128:(t + 1) * 128],
                                                 rhs=sT[:, k, :], start=(k == 0), stop=(k == 12))
                            nc.scalar.activation(out=h1[:, t, :], in_=ph[:], func=AF.Relu,
                                                 bias=b1c_t[:, t:t + 1])
                        h1q.append(h1)
                    psc = ps4r.tile([128, 512], f32, name="psc")
                    for j in range(2):
                        for t in range(8):
                            nc.tensor.matmul(out=psc[:],
                                             lhsT=w2r_t[:, j * 1024 + t * 128:
                                                        j * 1024 + (t + 1) * 128],
                                             rhs=h1q[j][:, t, :],
                                             start=(j == 0 and t == 0),
                                             stop=(j == 1 and t == 7))
                    w0b = sb4s.tile([128, 512], bf16, name="w0b")
                    nc.scalar.activation(out=w0b[:], in_=psc[:], func=AF.Sigmoid)
                    # in-place gate combine: sT0 <- sT1 + w0*(sT0-sT1)
                    sTc = sTq[0]
                    nc.vector.tensor_tensor(out=sTc[:], in0=sTc[:], in1=sTq[1][:],
                                            op=OP.subtract)
                    nc.vector.tensor_tensor(
                        out=sTc[:], in0=sTc[:],
                        in1=w0b[:].unsqueeze(1).broadcast_to([128, 13, 512]),
                        op=OP.mult)
                    nc.vector.tensor_tensor(out=sTc[:], in0=sTc[:], in1=sTq[1][:], op=OP.add)
                    for b in range(4):
                        m = q * 4 + b
                        stgh = sb4s.tile([128, 776], bf16, name="stgh")
                        for (n0, nw) in [(0, 512), (512, 264)]:
                            pk = ps4.tile([128, 512], f32, name="pk")
                            for k in range(13):
                                nc.tensor.matmul(out=pk[:, :nw],
                                                 lhsT=sTc[:, k, b * 128:(b + 1) * 128],
                                                 rhs=wk_t[:, k, n0:n0 + nw],
                                                 start=(k == 0), stop=(k == 12))
                            nc.scalar.activation(out=stgh[:, n0:n0 + nw], in_=pk[:, :nw],
                                                 func=AF.Copy)
                        nc.sync.dma_start(out=tk_in[m * 128:(m + 1) * 128, :776], in_=stgh[:])
                    nc.gpsimd.collective_compute(
                        "AllGather", OP.bypass, replica_groups=RG,
                        ins=[tk_in[q * 512:(q + 1) * 512, :]],
                        outs=[tk_tab[q * 4096:(q + 1) * 4096, :]])

            # -------- P5b: sd_q = tok_cons @ Wq_ad --------------------------
            if PH < 5: break
            with tc.tile_pool(name=f"p5b_{it}", bufs=1) as sb5b, \
                 tc.tile_pool(name=f"p5bp_{it}", bufs=2, space="PSUM") as ps5b:
                tokcT = kp2.tile([128, 6, NTL], bf16, name="tokcT")
                if it == 0:
                    nc.sync.dma_start(out=tokcT[:],
                                      in_=tokTp[:].rearrange("(k p) e -> p k e", p=128))
                else:
                    nc.sync.dma_start_transpose(out=tokcT[:], in_=tkc_dram[:])
                wq_t = sb5b.tile([128, 6, 8], bf16, name="wqt")
                nc.sync.dma_start(out=wq_t[:], in_=wq_ad[:].rearrange("(k p) n -> p k n", p=128))
                sd_q = kp1.tile([128, WCT, 8], bf16, name=f"sd_q{it}")
                for m in range(WCT):
                    pq = ps5b.tile([128, 16], f32, name="pq")
                    for k in range(6):
                        nc.tensor.matmul(out=pq[:, :8], lhsT=tokcT[:, k, m * 128:(m + 1) * 128],
                                         rhs=wq_t[:, k, :], start=(k == 0), stop=(k == 5))
                    nc.scalar.activation(out=sd_q[:, m, :], in_=pq[:, :8], func=AF.Copy)

            # -------- P6: ct GAT -> tok_cons --------------------------------
            if PH < 6: break
            with tc.tile_pool(name=f"p6_{it}", bufs=2) as sb6, \
                 tc.tile_pool(name=f"p6pv_{it}", bufs=2, space="PSUM") as ps6a, \
                 tc.tile_pool(name=f"p6ps_{it}", bufs=2, space="PSUM") as ps6b:

                tkw_tiles = {}

                def wb_tok(w, hh, pv, rec, sb):
                    if hh == 0:
                        tkw_tiles[w] = sb.tile([128, DH], f32, name="tkw")
                    tkw = tkw_tiles[w]
                    nc.vector.tensor_tensor(
                        out=tkw[:, hh * 384:(hh + 1) * 384].rearrange("p (d h) -> p d h", h=8),
                        in0=pv[:, :384].rearrange("p (d h) -> p d h", h=8),
                        in1=rec[:].unsqueeze(1).broadcast_to([128, 48, 8]),
                        op=OP.mult)

                def wb_tok_fin(w, sb):
                    tkw = tkw_tiles.pop(w)
                    if it == NITER - 1:
                        nc.sync.dma_start(out=tcf_dram[w * 128:(w + 1) * 128, :], in_=tkw[:])
                    stg = sb.tile([128, DH], bf16, name="stgt")
                    nc.vector.tensor_copy(out=stg[:], in_=tkw[:])
                    nc.sync.dma_start(out=tkc_dram[w * 128:(w + 1) * 128, :], in_=stg[:])
                    if it == 0:
                        stg_nat = sb.tile([128, DH], bf16, name="stgn")
                        nc.vector.tensor_copy(
                            out=stg_nat[:].rearrange("p (h d) -> p h d", h=8),
                            in_=stg[:].rearrange("p (d h) -> p h d", h=8))
                        nc.sync.dma_start(out=tt_in[w * 128:(w + 1) * 128, :], in_=stg_nat[:])

                def wb_tok_fin2(w, sb):
                    wb_tok_fin(w, sb)
                    if it == 0 and NITER > 1 and w in (3, 7):
                        hof = (w // 4) * 512
                        nc.gpsimd.collective_compute(
                            "AllGather", OP.bypass, replica_groups=RG,
                            ins=[tt_in[hof:hof + 512, :]],
                            outs=[tt_tab[hof * NCORES:(hof + 512) * NCORES, :]])

                gat_pass(sb6, ps6a, ps6b, tk_tab, SWK, idx_ct_t, oh_ct_d, ohd_ct_d,
                         sd_q, WCT, C_CT, DH, blk0=0, writeback=wb_tok,
                         wb_final=wb_tok_fin2, tag="ct")

        # -------- P7: fuse gate + blend -------------------------------------
        if PH >= 7:
          with tc.tile_pool(name="p7", bufs=1) as sb7, \
              tc.tile_pool(name="p7s", bufs=3) as sb7s, \
              tc.tile_pool(name="p7p", bufs=2, space="PSUM") as ps7:
             fw_t = sb7.tile([128, 12, DH], bf16, name="fwt")
             nc.sync.dma_start(out=fw_t[:], in_=fuse_w_d[:].rearrange("(k p) n -> p k n", p=128))
             fb_t = sb7.tile([1, DH], bf16, name="fbt")
             nc.sync.dma_start(out=fb_t[:], in_=fb_row_d[:])
             tokT_t = sb7.tile([128, 6, NTL], bf16, name="tokTt")
             nc.sync.dma_start(out=tokT_t[:], in_=tokT[:].rearrange("(k p) e -> p k e", p=128))
             tokcT_f = sb7.tile([128, 6, NTL], bf16, name="tokcTf")
             nc.sync.dma_start_transpose(out=tokcT_f[:], in_=tkc_dram[:])
             for m in range(WCT):
                 f_t = sb7s.tile([128, DH], f32, name="f_t")
                 for (n0, nw) in [(0, 512), (512, 256)]:
                     pf = ps7.tile([128, 512], f32, name="pf")
                     for k in range(12):
                         lt = tokT_t[:, k, m * 128:(m + 1) * 128] if k < 6 else \
                              tokcT_f[:, k - 6, m * 128:(m + 1) * 128]
                         nc.tensor.matmul(out=pf[:, :nw], lhsT=lt, rhs=fw_t[:, k, n0:n0 + nw],
                                          start=(k == 0), stop=False)
                     nc.tensor.matmul(out=pf[:, :nw], lhsT=ones1_t[:], rhs=fb_t[:, n0:n0 + nw],
                                      start=False, stop=True)
                     nc.scalar.activation(out=f_t[:, n0:n0 + nw], in_=pf[:, :nw], func=AF.Sigmoid)
                 tok_in = sb7s.tile([128, DH], f32, name="tok_in")
                 nc.sync.dma_start(out=tok_in[:], in_=tok_f32_d[m * 128:(m + 1) * 128, :])
                 tcf_in = sb7s.tile([128, DH], f32, name="tcf_in")
                 nc.sync.dma_start(out=tcf_in[:], in_=tcf_dram[m * 128:(m + 1) * 128, :])
                 dlt = sb7s.tile([128, DH], f32, name="dlt")
                 nc.vector.tensor_tensor(out=dlt[:], in0=tok_in[:], in1=tcf_in[:], op=OP.subtract)
                 nc.vector.tensor_tensor(out=dlt[:], in0=f_t[:], in1=dlt[:], op=OP.mult)
                 nc.vector.tensor_tensor(out=dlt[:], in0=tcf_in[:], in1=dlt[:], op=OP.add)
                 udlt = sb7s.tile([128, DH], f32, name="udlt")
                 nc.vector.tensor_copy(
                     out=udlt[:].rearrange("p (h d) -> p h d", h=8),
                     in_=dlt[:].rearrange("p (d h) -> p h d", h=8))
                 nc.sync.dma_start(out=out_d[m * 128:(m + 1) * 128, :], in_=udlt[:])

    nc.finalize()
    return nc


# ------------------------------------------------------------------ driver --

def run(inputs, trace=False):
    in_maps, C_CC, C_CT = _host_prep(inputs)
    nc = _build_nc(C_CC, C_CT)
    res = run_bass_kernel_spmd(nc, in_maps, core_ids=list(range(NCORES)), trace=trace)
    out = np.concatenate([res.results[c]["out"] for c in range(NCORES)], axis=0)
    return out.astype(np.float32), res


def kernel(**inputs) -> np.ndarray:
    out, _ = run(inputs, trace=False)
    return out


# revision 13
# speedup vs baseline: 1.2523x; 1.0045x over previous
"""Trainium2 Bass kernel for nn_CorefModel (GNN message passing, 8 NeuronCores).

Sharding: constituents 2048/core, tokens 1024/core (row shards). Per-iteration
node-feature tables (hf, hb, hk) are produced by per-core GEMMs, AllGathered
into a full per-core HBM copy, and edge gathers (dma_gather) read full rows
from the local copy. GAT edge aggregation runs as one-hot scatter matmuls on
the TensorEngine over destination-sorted edge chunks; the softmax denominator
is accumulated by the same one-hot against exp values.

Perf structure vs v1:
- Feature blocks stored HEAD-MINOR ([dh, H] interleaved) so the per-edge
  alpha broadcast-multiply hits the DVE 2x packed mode (inner step-1 dim).
  Weight matrices are permuted on the host to match (columns for producers,
  rows for consumers); the final output is un-permuted on device.
- One-hot scatter planes (edge->dst and dst->edge) are precomputed on the
  host and streamed from HBM instead of being built with is_equal on DVE.
- cc-GAT passes run (fwd_j, bwd_j) per order j: single live order
  accumulator, no SBUF spill/reload dance.
- Order panels are spilled untransposed and reloaded via xbar DMA transpose
  (replaces hundreds of PE transposes + ACT copies).
- P4 computes attention scores in transposed [1, cons] form (w2 as lhsT),
  gate-combines the two order panels on DVE, then runs a single Wk GEMM.
Compute dtype bf16 with fp32 PSUM accumulation; final blend in fp32.
"""
import sys, os
sys.path.insert(0, '/opt/trn_rl_repo')
import math
import numpy as np
import ml_dtypes

import concourse.bass as bass
from concourse import bacc
import concourse.tile as tile
from concourse import mybir
from concourse.bass_utils import run_bass_kernel_spmd

BF = ml_dtypes.bfloat16
F32 = np.float32

H = 8
NT, NC = 8192, 16384
DH, DCH = 768, 128
D = 2 * DH + DCH          # 1664
NCORES = 8
NCL, NTL = NC // NCORES, NT // NCORES     # 2048, 1024
WCC, WCT = NCL // 128, NTL // 128         # 16, 8
SWF = 1792                 # cc table row stride (bf16 elems, 3584B)
SWK = 896                  # hk table row stride (1792B)

f32 = mybir.dt.float32
bf16 = mybir.dt.bfloat16
i16 = mybir.dt.int16
AF = mybir.ActivationFunctionType
OP = mybir.AluOpType


# ---------------------------------------------------------------- host prep --

def _bdiag(a):
    h, dh = a.shape
    m = np.zeros((h * dh, h), F32)
    for i in range(h):
        m[i * dh:(i + 1) * dh, i] = a[i]
    return m


def _permC(W, h, dh):
    # natural col f = hh*dh+d  ->  head-minor col p = d*h+hh
    r = W.shape[0]
    return np.ascontiguousarray(W.reshape(r, h, dh).transpose(0, 2, 1).reshape(r, h * dh))


def _permR(W, h, dh):
    c = W.shape[1]
    return np.ascontiguousarray(W.reshape(h, dh, c).transpose(1, 0, 2).reshape(h * dh, c))


def _permV(v, h, dh):
    return np.ascontiguousarray(v.reshape(h, dh).T.reshape(h * dh))


def _idx_plane(idxs):
    """[128, n/16] int16: idx i at (i%16, i//16), replicated across 8 Q7 groups."""
    n = len(idxs)
    plane = np.zeros((128, n // 16), np.int16)
    plane[np.arange(n) % 16, np.arange(n) // 16] = idxs
    for g in range(1, 8):
        plane[16 * g:16 * (g + 1)] = plane[:16]
    return plane


def _prep_edges(srcs, dsts, wins):
    """Per edge set: sort by dst, split into per-(core, 128-dst-window) groups.
    Returns uniform chunks/window C and per-core index + one-hot planes:
      oh  [128(e), blocks*C*128] : oh[e, (blk,c,j)] = 1 iff dstloc(edge c*128+e)==j
      ohd [128(j), blocks*C*128] : ohd[j, (blk,c,e)] = 1 iff dstloc(edge c*128+e)==j
    """
    nsets = len(srcs)
    percw = [[[None] * wins for _ in range(nsets)] for _ in range(NCORES)]
    maxe = 0
    for s in range(nsets):
        src, dst = srcs[s], dsts[s]
        gwin = dst // 128
        order = np.argsort(gwin, kind='stable')
        src_s, dst_s, gwin_s = src[order], dst[order], gwin[order]
        bounds = np.searchsorted(gwin_s, np.arange(NCORES * wins + 1))
        for c in range(NCORES):
            for w in range(wins):
                g = c * wins + w
                lo, hi = bounds[g], bounds[g + 1]
                maxe = max(maxe, hi - lo)
                percw[c][s][w] = (src_s[lo:hi], dst_s[lo:hi] % 128)
    C = max(1, math.ceil(maxe / 128))
    E = C * 128
    ar128 = np.arange(128)
    out = []
    for c in range(NCORES):
        nblk = nsets * wins
        idxp = np.zeros((128, nblk * (E // 16)), np.int16)
        oh = np.zeros((128, nblk * E), BF)
        ohd = np.zeros((128, nblk * E), BF)
        for s in range(nsets):
            for w in range(wins):
                src_e, dloc = percw[c][s][w]
                k = len(src_e)
                blk = s * wins + w
                sp = np.zeros(E, np.int16); sp[:k] = src_e.astype(np.int16)
                dp = np.full(E, -1, np.int64); dp[:k] = dloc
                idxp[:, blk * (E // 16):(blk + 1) * (E // 16)] = _idx_plane(sp)
                eq = (dp.reshape(C, 128)[:, :, None] == ar128[None, None, :])  # [c,e,j]
                oh[:, blk * E:(blk + 1) * E] = \
                    eq.transpose(1, 0, 2).reshape(128, E).astype(BF)
                ohd[:, blk * E:(blk + 1) * E] = \
                    eq.transpose(2, 0, 1).reshape(128, E).astype(BF)
        out.append(dict(idx=idxp, oh=oh, ohd=ohd))
    return C, out


def _host_prep(inp):
    tok = np.asarray(inp['token_embeddings'], F32)
    starts = np.asarray(inp['constituent_starts']).astype(np.int64)
    ends = np.asarray(inp['constituent_ends']).astype(np.int64)
    labels = np.asarray(inp['constituent_labels']).astype(np.int64)
    label_emb = np.asarray(inp['cons_type_table'], F32)[labels]        # [NC, 128]

    Wf = np.asarray(inp['Wf'], F32); Wb = np.asarray(inp['Wb'], F32)
    Wk = np.asarray(inp['Wk'], F32); Wq = np.asarray(inp['Wq'], F32)
    # tables: columns head-minor; Wf/Wb rows stay natural (consume cons),
    # Wk rows permuted (consume fused order features).
    Wf_ext = np.concatenate([_permC(Wf, 8, 208),
                             Wf @ _bdiag(np.asarray(inp['af_s'], F32)),
                             Wf @ _bdiag(np.asarray(inp['af_d'], F32))], 1).astype(BF)
    Wb_ext = np.concatenate([_permC(Wb, 8, 208),
                             Wb @ _bdiag(np.asarray(inp['ab_s'], F32)),
                             Wb @ _bdiag(np.asarray(inp['ab_d'], F32))], 1).astype(BF)
    Wk_ext = _permR(np.concatenate([_permC(Wk, 8, 96),
                                    Wk @ _bdiag(np.asarray(inp['act_s'], F32))], 1),
                    8, 208).astype(BF)
    Wq_ad = _permR(Wq @ _bdiag(np.asarray(inp['act_d'], F32)), 8, 96).astype(BF)
    w1 = _permR(np.asarray(inp['attn_w1'], F32), 8, 208).astype(BF)    # [1664, 1024]
    w2c = np.asarray(inp['attn_w2'], F32).reshape(8, 128).T.copy()
    w2r = np.concatenate([np.repeat(w2c, 128, axis=1),
                          np.repeat(-w2c, 128, axis=1)], 1).astype(BF)  # [128, 2*8*128]
    b1c = np.asarray(inp['attn_b1'], F32).reshape(8, 128).T.copy()     # [128, 8] f32
    fw = np.asarray(inp['fuse_w'], F32)
    fuse_w = np.concatenate([fw[:DH], _permR(fw[DH:], 8, 96)], 0)
    fuse_w = _permC(fuse_w, 8, 96).astype(BF)                          # [1536, 768]
    fb_row = _permV(np.asarray(inp['fuse_b'], F32), 8, 96).reshape(1, DH).astype(BF)

    cons = np.concatenate([tok[starts], tok[ends], label_emb], 1)      # [NC, 1664] f32

    cc_src = np.asarray(inp['cc_src']); cc_dst = np.asarray(inp['cc_dst'])
    ct_src = np.asarray(inp['ct_src']); ct_dst = np.asarray(inp['ct_dst'])

    def _remap512(g, local):
        # table rows after 512-row piecewise AllGathers: piece p holds local
        # rows p*512:(p+1)*512 of every rank at tab[p*4096 + r*512 + (l%512)].
        r = g // local
        l = g % local
        return (l // 512) * (NCORES * 512) + r * 512 + (l % 512)
    cc_src = _remap512(cc_src, NCL)
    ct_src = _remap512(ct_src, NCL)
    starts_t = _remap512(starts, NTL)
    ends_t = _remap512(ends, NTL)
    C_CC, cc_meta = _prep_edges([cc_src[s] for s in range(4)], [cc_dst[s] for s in range(4)], WCC)
    C_CT, ct_meta = _prep_edges([ct_src], [ct_dst], WCT)

    ident = np.eye(128, dtype=F32).astype(BF)
    ones1 = np.ones((1, 128), F32).astype(BF)

    in_maps = []
    for c in range(NCORES):
        csl = slice(c * NCL, (c + 1) * NCL)
        tsl = slice(c * NTL, (c + 1) * NTL)
        m = dict(
            consT0=np.ascontiguousarray(cons[csl].T).astype(BF),       # [1664, 2048]
            labelT=np.ascontiguousarray(label_emb[csl].T).astype(BF),  # [128, 2048]
            tokT=np.ascontiguousarray(tok[tsl].T).astype(BF),          # [768, 1024]
            tokTp=np.ascontiguousarray(_permC(tok[tsl], 8, 96).T).astype(BF),
            tok_f32=_permC(tok[tsl], 8, 96),                           # [1024, 768] permuted
            idx_starts=_idx_plane(starts_t[csl].astype(np.int16)),     # [128, 128]
            idx_ends=_idx_plane(ends_t[csl].astype(np.int16)),
            wf_ext=Wf_ext, wb_ext=Wb_ext, wk_ext=Wk_ext, wq_ad=Wq_ad,
            w1=w1, w2r=w2r, b1c=b1c, fuse_w=fuse_w, fb_row=fb_row,
            ident=ident, ones1=ones1,
            idx_cc=cc_meta[c]['idx'], oh_cc=cc_meta[c]['oh'], ohd_cc=cc_meta[c]['ohd'],
            idx_ct=ct_meta[c]['idx'], oh_ct=ct_meta[c]['oh'], ohd_ct=ct_meta[c]['ohd'],
        )
        in_maps.append(m)
    return in_maps, C_CC, C_CT


# ------------------------------------------------------------- device build --

def _build_nc(C_CC, C_CT):
    ECC = C_CC * 128
    ECT = C_CT * 128
    nc = bacc.Bacc("TRN2", num_devices=NCORES)

    def ein(name, shape, dt_):
        return nc.dram_tensor(name, shape, dt_, kind="ExternalInput")

    consT0 = ein("consT0", [D, NCL], bf16)
    labelT = ein("labelT", [128, NCL], bf16)
    tokT = ein("tokT", [DH, NTL], bf16)
    tokTp = ein("tokTp", [DH, NTL], bf16)
    tok_f32_d = ein("tok_f32", [NTL, DH], f32)
    idx_starts = ein("idx_starts", [128, NCL // 16], i16)
    idx_ends = ein("idx_ends", [128, NCL // 16], i16)
    wf_ext = ein("wf_ext", [D, 1680], bf16)
    wb_ext = ein("wb_ext", [D, 1680], bf16)
    wk_ext = ein("wk_ext", [D, 776], bf16)
    wq_ad = ein("wq_ad", [DH, 8], bf16)
    w1_d = ein("w1", [D, 1024], bf16)
    w2r_d = ein("w2r", [128, 2048], bf16)
    b1c_d = ein("b1c", [128, 8], f32)
    fuse_w_d = ein("fuse_w", [2 * DH, DH], bf16)
    fb_row_d = ein("fb_row", [1, DH], bf16)
    ident_d = ein("ident", [128, 128], bf16)
    ones1_d = ein("ones1", [1, 128], bf16)
    idx_cc_d = ein("idx_cc", [128, 4 * WCC * (ECC // 16)], i16)
    oh_cc_d = ein("oh_cc", [128, 4 * WCC * ECC], bf16)
    ohd_cc_d = ein("ohd_cc", [128, 4 * WCC * ECC], bf16)
    idx_ct_d = ein("idx_ct", [128, WCT * (ECT // 16)], i16)
    oh_ct_d = ein("oh_ct", [128, WCT * ECT], bf16)
    ohd_ct_d = ein("ohd_ct", [128, WCT * ECT], bf16)

    out_d = nc.dram_tensor("out", [NTL, DH], f32, kind="ExternalOutput")

    tf_in = nc.dram_tensor("tf_in", [NCL, SWF], bf16)
    tb_in = nc.dram_tensor("tb_in", [NCL, SWF], bf16)
    tk_in = nc.dram_tensor("tk_in", [NCL, SWK], bf16)
    tt_in = nc.dram_tensor("tt_in", [NTL, DH], bf16)
    tf_tab = nc.dram_tensor("tf_tab", [NC, SWF], bf16, addr_space="Shared")
    tb_tab = nc.dram_tensor("tb_tab", [NC, SWF], bf16, addr_space="Shared")
    tk_tab = nc.dram_tensor("tk_tab", [NC, SWK], bf16, addr_space="Shared")
    tt_tab = nc.dram_tensor("tt_tab", [NT, DH], bf16, addr_space="Shared")
    ord_dram = [[nc.dram_tensor(f"ordp{j}_{q}", [512, D], bf16) for q in range(4)]
                for j in range(2)]
    tkc_dram = nc.dram_tensor("tkc", [NTL, DH], bf16)
    tcf_dram = nc.dram_tensor("tcf", [NTL, DH], f32)

    RG = [list(range(NCORES))]

    with tile.TileContext(nc) as tc:
      with tc.tile_pool(name="const", bufs=1) as cp, \
           tc.tile_pool(name="keep1", bufs=1) as kp1, \
           tc.tile_pool(name="keep2", bufs=2) as kp2:
        def cload(name, dram, shape, dt_):
            t = cp.tile(shape, dt_, name=name)
            nc.sync.dma_start(out=t[:], in_=dram[:])
            return t
        ident_t = cload("ident", ident_d, [128, 128], bf16)
        ones1_t = cload("ones1", ones1_d, [1, 128], bf16)
        idx_cc_t = cload("idx_cc", idx_cc_d, [128, 4 * WCC * (ECC // 16)], i16)
        idx_ct_t = cload("idx_ct", idx_ct_d, [128, WCT * (ECT // 16)], i16)
        w2r_t = cload("w2r", w2r_d, [128, 2048], bf16)
        b1c_t = cload("b1c", b1c_d, [128, 8], f32)

        def gemm_block(ps, lhsT_fn, nkt, w_t, nchunks, stg, tag):
            for (n0, nw) in nchunks:
                pt = ps.tile([128, 512], f32, name=f"gp{tag}")
                for k in range(nkt):
                    nc.tensor.matmul(out=pt[:, :nw], lhsT=lhsT_fn(k),
                                     rhs=w_t[:, k, n0:n0 + nw],
                                     start=(k == 0), stop=(k == nkt - 1))
                nc.scalar.activation(out=stg[:, n0:n0 + nw], in_=pt[:, :nw], func=AF.Copy)

        def gat_pass(sb, ps_pv, ps_sm, tab, stride, idx_t, oh_d, ohd_d, sd_tile,
                     wins, C, dfeat, blk0, writeback, wb_final=lambda w, sb: None, tag=""):
            E = C * 128
            dh = dfeat // 8
            half = dfeat // 2
            hchunks = []
            n0 = 0
            while n0 < half:
                nw = min(512, half - n0)
                hchunks.append((n0, nw)); n0 += nw
            pvsz = 1024 if half > 512 else 512
            gstep = 5
            for w in range(wins):
                blk = blk0 + w
                segs = []
                for c0 in range(0, C, gstep):
                    cc = min(gstep, C - c0)
                    ioff = blk * (E // 16) + c0 * 8
                    gt = sb.tile([128, cc, stride], bf16, name=f"gw{tag}{c0}")
                    nc.gpsimd.dma_gather(
                        out_ap=gt[:, 0:cc, :], in_ap=tab[:],
                        idxs_ap=idx_t[:, ioff:ioff + cc * 8],
                        num_idxs=cc * 128, num_idxs_reg=cc * 128, elem_size=stride)
                    segs.append((c0, cc, gt))

                def gch(c):
                    for (c0, cc, gt) in segs:
                        if c0 <= c < c0 + cc:
                            return gt[:, c - c0, :]
                ohw = sb.tile([128, C, 128], bf16, name=f"oh{tag}")
                nc.sync.dma_start(out=ohw[:],
                                  in_=oh_d[:, blk * E:(blk + 1) * E]
                                      .rearrange("p (c j) -> p c j", c=C))
                ohdw = sb.tile([128, C, 128], bf16, name=f"ohd{tag}")
                nc.sync.dma_start(out=ohdw[:],
                                  in_=ohd_d[:, blk * E:(blk + 1) * E]
                                      .rearrange("p (c e) -> p c e", c=C))
                strip = ps_sm.tile([128, 512], f32, name=f"strip{tag}")
                for c in range(C):
                    nc.tensor.matmul(out=strip[:, c * 8:(c + 1) * 8], lhsT=ohdw[:, c, :],
                                     rhs=sd_tile[:, w, :], start=True, stop=False)
                    nc.tensor.matmul(out=strip[:, c * 8:(c + 1) * 8], lhsT=ident_t[:],
                                     rhs=gch(c)[:, dfeat:dfeat + 8], start=False, stop=True)
                lg = sb.tile([128, C * 8], f32, name=f"lg{tag}")
                nc.scalar.activation(out=lg[:], in_=strip[:, :C * 8], func=AF.Prelu, alpha=0.2)
                ex = sb.tile([128, C, 8], bf16, name=f"ex{tag}")
                nc.scalar.activation(out=ex[:].rearrange("p c e -> p (c e)"), in_=lg[:],
                                     func=AF.Exp)
                rss = []
                for c in range(C):
                    rs = sb.tile([128, dfeat], bf16, name=f"rs{tag}")
                    nc.vector.tensor_tensor(
                        out=rs[:].rearrange("p (d h) -> p d h", h=8),
                        in0=gch(c)[:, :dfeat].rearrange("p (d h) -> p d h", h=8),
                        in1=ex[:, c, :].unsqueeze(1).broadcast_to([128, dh, 8]),
                        op=OP.mult)
                    rss.append(rs)
                rec = None
                for hh in range(2):
                    pv = ps_pv.tile([128, pvsz], f32, name=f"pv{tag}")
                    for c in range(C):
                        for (n0, nw) in hchunks:
                            nc.tensor.matmul(out=pv[:, n0:n0 + nw], lhsT=ohw[:, c, :],
                                             rhs=rss[c][:, hh * half + n0:hh * half + n0 + nw],
                                             start=(c == 0), stop=(c == C - 1))
                        if hh == 0:
                            nc.tensor.matmul(out=strip[:, 384:392], lhsT=ohw[:, c, :],
                                             rhs=ex[:, c, :], start=(c == 0), stop=(c == C - 1))
                    if hh == 0:
                        dent = sb.tile([128, 8], f32, name=f"dent{tag}")
                        nc.vector.tensor_scalar(out=dent[:], in0=strip[:, 384:392],
                                                scalar1=1e-9, scalar2=None, op0=OP.add)
                        rec = sb.tile([128, 8], f32, name=f"rec{tag}")
                        nc.vector.reciprocal(out=rec[:], in_=dent[:])
                    writeback(w, hh, pv, rec, sb)
                wb_final(w, sb)

        # ============================ iterations ============================
        PH = int(os.environ.get("PHASE_LIMIT", "99"))
        NITER = int(os.environ.get("NITER", "2"))
        for it in range(NITER):
            # -------- P1+P2: consT, hf/hb tables, AllGather -----------------
            with tc.tile_pool(name=f"p2_{it}", bufs=1) as sb2, \
                 tc.tile_pool(name=f"p2s_{it}", bufs=3) as sb2s, \
                 tc.tile_pool(name=f"p2p_{it}", bufs=2, space="PSUM") as ps2:
                consT = sb2.tile([128, 13, NCL], bf16, name="consT")
                if it == 0:
                    nc.sync.dma_start(out=consT[:],
                                      in_=consT0[:].rearrange("(k p) e -> p k e", p=128))
                else:
                    ist = sb2.tile([128, NCL // 16], i16, name="ist")
                    nc.sync.dma_start(out=ist[:], in_=idx_starts[:])
                    ien = sb2.tile([128, NCL // 16], i16, name="ien")
                    nc.sync.dma_start(out=ien[:], in_=idx_ends[:])
                    for half, idxt in ((0, ist), (1, ien)):
                        for q in range(4):
                            gt = sb2s.tile([128, 6, 512], bf16, name="gtc")
                            nc.gpsimd.dma_gather(
                                out_ap=gt[:], in_ap=tt_tab[:],
                                idxs_ap=idxt[:, q * 32:(q + 1) * 32],
                                num_idxs=512, num_idxs_reg=512,
                                elem_size=DH, transpose=True)
                            nc.vector.tensor_copy(
                                out=consT[:, 6 * half:6 * half + 6, q * 512:(q + 1) * 512],
                                in_=gt[:])
                    nc.sync.dma_start(out=consT[:, 12, :], in_=labelT[:])
                sd_f = kp1.tile([128, WCC, 8], bf16, name=f"sd_f{it}")
                sd_b = kp1.tile([128, WCC, 8], bf16, name=f"sd_b{it}")
                for (wd, outd, tabd, sdt) in [(wf_ext, tf_in, tf_tab, sd_f),
                                              (wb_ext, tb_in, tb_tab, sd_b)]:
                    w_t = sb2.tile([128, 13, 1680], bf16, name="wtab")
                    nc.sync.dma_start(out=w_t[:], in_=wd[:].rearrange("(k p) n -> p k n", p=128))
                    for m in range(WCC):
                        stg = sb2s.tile([128, 1680], bf16, name="stg")
                        gemm_block(ps2, lambda k: consT[:, k, m * 128:(m + 1) * 128], 13,
                                   w_t, [(0, 512), (512, 512), (1024, 512), (1536, 144)],
                                   stg, "t")
                        nc.sync.dma_start(out=outd[m * 128:(m + 1) * 128, :1680], in_=stg[:])
                        nc.vector.tensor_copy(out=sdt[:, m, :], in_=stg[:, 1672:1680])
                        if m % 4 == 3:
                            q = m // 4
                            nc.gpsimd.collective_compute(
                                "AllGather", OP.bypass, replica_groups=RG,
                                ins=[outd[q * 512:(q + 1) * 512, :]],
                                outs=[tabd[q * 4096:(q + 1) * 4096, :]])

            # -------- P5b: sd_q = tok_cons @ Wq_ad --------------------------
            with tc.tile_pool(name=f"p5b_{it}", bufs=1) as sb5b, \
                 tc.tile_pool(name=f"p5bp_{it}", bufs=2, space="PSUM") as ps5b:
                tokcT = kp2.tile([128, 6, NTL], bf16, name="tokcT")
                if it == 0:
                    nc.sync.dma_start(out=tokcT[:],
                                      in_=tokTp[:].rearrange("(k p) e -> p k e", p=128))
                else:
                    nc.sync.dma_start_transpose(out=tokcT[:], in_=tkc_dram[:])
                wq_t = sb5b.tile([128, 6, 8], bf16, name="wqt")
                nc.sync.dma_start(out=wq_t[:], in_=wq_ad[:].rearrange("(k p) n -> p k n", p=128))
                sd_q = kp1.tile([128, WCT, 8], bf16, name=f"sd_q{it}")
                for m in range(WCT):
                    pq = ps5b.tile([128, 16], f32, name="pq")
                    for k in range(6):
                        nc.tensor.matmul(out=pq[:, :8], lhsT=tokcT[:, k, m * 128:(m + 1) * 128],
                                         rhs=wq_t[:, k, :], start=(k == 0), stop=(k == 5))
                    nc.scalar.activation(out=sd_q[:, m, :], in_=pq[:, :8], func=AF.Copy)

            # -------- P3: cc GATs -> order_j, plain spill to ord_dram -------
            if PH < 3: break
            with tc.tile_pool(name=f"p3_{it}", bufs=1) as sb3o, \
                 tc.tile_pool(name=f"p3s_{it}", bufs=4) as sb3, \
                 tc.tile_pool(name=f"p3pv_{it}", bufs=2, space="PSUM") as ps3a, \
                 tc.tile_pool(name=f"p3ps_{it}", bufs=2, space="PSUM") as ps3b:

                def mk_wb_first(ordt):
                    def wb_first(w, hh, pv, rec, sb):
                        nc.vector.tensor_tensor(
                            out=ordt[:, w, hh * 832:(hh + 1) * 832]
                                .rearrange("p (d h) -> p d h", h=8),
                            in0=pv[:, :832].rearrange("p (d h) -> p d h", h=8),
                            in1=rec[:].unsqueeze(1).broadcast_to([128, 104, 8]),
                            op=OP.mult)
                    return wb_first

                def mk_wb_add(ordt):
                    def wb_add(w, hh, pv, rec, sb):
                        t = sb.tile([128, 832], bf16, name="tadd")
                        nc.vector.tensor_tensor(
                            out=t[:].rearrange("p (d h) -> p d h", h=8),
                            in0=pv[:, :832].rearrange("p (d h) -> p d h", h=8),
                            in1=rec[:].unsqueeze(1).broadcast_to([128, 104, 8]),
                            op=OP.mult)
                        nc.vector.tensor_tensor(
                            out=ordt[:, w, hh * 832:(hh + 1) * 832],
                            in0=ordt[:, w, hh * 832:(hh + 1) * 832],
                            in1=t[:], op=OP.add)
                    return wb_add

                def mk_wb_fin(ordt, j):
                    def wb_fin(w, sb):
                        nc.sync.dma_start(
                            out=ord_dram[j][w // 4][(w % 4) * 128:(w % 4 + 1) * 128, :],
                            in_=ordt[:, w, :])
                    return wb_fin

                ordt0 = sb3o.tile([128, WCC, D], bf16, name="ordt")
                gat_pass(sb3, ps3a, ps3b, tf_tab, SWF, idx_cc_t, oh_cc_d, ohd_cc_d,
                         sd_f, WCC, C_CC, D, blk0=0,
                         writeback=mk_wb_first(ordt0), tag="cc")
                for q in range(4):
                    nc.sync.dma_start(
                        out=ord_dram[0][q][:].rearrange("(w p) d -> p w d", p=128),
                        in_=ordt0[:, q * 4:(q + 1) * 4, :])
                ordt1 = sb3o.tile([128, WCC, D], bf16, name="ordt")
                gat_pass(sb3, ps3a, ps3b, tf_tab, SWF, idx_cc_t, oh_cc_d, ohd_cc_d,
                         sd_f, WCC, C_CC, D, blk0=WCC,
                         writeback=mk_wb_first(ordt1), tag="cc")
                gat_pass(sb3, ps3a, ps3b, tb_tab, SWF, idx_cc_t, oh_cc_d, ohd_cc_d,
                         sd_b, WCC, C_CC, D, blk0=3 * WCC,
                         writeback=mk_wb_add(ordt1), wb_final=mk_wb_fin(ordt1, 1),
                         tag="cc")
                ordt0b = sb3o.tile([128, WCC, D], bf16, name="ordt")
                for q in range(4):
                    nc.sync.dma_start(
                        out=ordt0b[:, q * 4:(q + 1) * 4, :],
                        in_=ord_dram[0][q][:].rearrange("(w p) d -> p w d", p=128))
                gat_pass(sb3, ps3a, ps3b, tb_tab, SWF, idx_cc_t, oh_cc_d, ohd_cc_d,
                         sd_b, WCC, C_CC, D, blk0=2 * WCC,
                         writeback=mk_wb_add(ordt0b), wb_final=mk_wb_fin(ordt0b, 0),
                         tag="cc")

            # -------- P4: attention scores + gate + single Wk GEMM ----------
            if PH < 4: break
            with tc.tile_pool(name=f"p4_{it}", bufs=1) as sb4, \
                 tc.tile_pool(name=f"p4s_{it}", bufs=2) as sb4s, \
                 tc.tile_pool(name=f"p4p_{it}", bufs=2, space="PSUM") as ps4, \
                 tc.tile_pool(name=f"p4pr_{it}", bufs=1, space="PSUM") as ps4r:
                w1_t = sb4.tile([128, 13, 1024], bf16, name="w1t")
                nc.sync.dma_start(out=w1_t[:], in_=w1_d[:].rearrange("(k p) n -> p k n", p=128))
                wk_t = sb4.tile([128, 13, 776], bf16, name="wkt")
                nc.sync.dma_start(out=wk_t[:], in_=wk_ext[:].rearrange("(k p) n -> p k n", p=128))
                for q in range(4):
                    sTq = []
                    h1q = []
                    for j in range(2):
                        sT = sb4s.tile([128, 13, 512], bf16, name=f"sT{j}")
                        nc.sync.dma_start_transpose(
                            out=sT[:], in_=ord_dram[j][q][:])
                        sTq.append(sT)
                        h1 = sb4s.tile([128, 8, 512], bf16, name=f"h1_{j}")
                        for t in range(8):
                            ph = ps4.tile([128, 512], f32, name="ph")
                            for k in range(13):
                                nc.tensor.matmul(out=ph[:], lhsT=w1_t[:, k, t * 128:(t + 1) * 128],
                                                 rhs=sT[:, k, :], start=(k == 0), stop=(k == 12))
                            nc.scalar.activation(out=h1[:, t, :], in_=ph[:], func=AF.Relu,
                                                 bias=b1c_t[:, t:t + 1])
                        h1q.append(h1)
                    psc = ps4r.tile([128, 512], f32, name="psc")
                    for j in range(2):
                        for t in range(8):
                            nc.tensor.matmul(out=psc[:],
                                             lhsT=w2r_t[:, j * 1024 + t * 128:
                                                        j * 1024 + (t + 1) * 128],
                                             rhs=h1q[j][:, t, :],
                                             start=(j == 0 and t == 0),
                                             stop=(j == 1 and t == 7))
                    w0b = sb4s.tile([128, 512], bf16, name="w0b")
                    nc.scalar.activation(out=w0b[:], in_=psc[:], func=AF.Sigmoid)
                    # in-place gate combine: sT0 <- sT1 + w0*(sT0-sT1)
                    sTc = sTq[0]
                    nc.vector.tensor_tensor(out=sTc[:], in0=sTc[:], in1=sTq[1][:],
                                            op=OP.subtract)
                    nc.vector.tensor_tensor(
                        out=sTc[:], in0=sTc[:],
                        in1=w0b[:].unsqueeze(1).broadcast_to([128, 13, 512]),
                        op=OP.mult)
                    nc.vector.tensor_tensor(out=sTc[:], in0=sTc[:], in1=sTq[1][:], op=OP.add)
                    for b in range(4):
                        m = q * 4 + b
                        stgh = sb4s.tile([128, 776], bf16, name="stgh")
                        for (n0, nw) in [(0, 512), (512, 264)]:
                            pk = ps4.tile([128, 512], f32, name="pk")
                            for k in range(13):
                                nc.tensor.matmul(out=pk[:, :nw],
                                                 lhsT=sTc[:, k, b * 128:(b + 1) * 128],
                                                 rhs=wk_t[:, k, n0:n0 + nw],
                                                 start=(k == 0), stop=(k == 12))
                            nc.scalar.activation(out=stgh[:, n0:n0 + nw], in_=pk[:, :nw],
                                                 func=AF.Copy)
                        nc.sync.dma_start(out=tk_in[m * 128:(m + 1) * 128, :776], in_=stgh[:])
                    nc.gpsimd.collective_compute(
                        "AllGather", OP.bypass, replica_groups=RG,
                        ins=[tk_in[q * 512:(q + 1) * 512, :]],
                        outs=[tk_tab[q * 4096:(q + 1) * 4096, :]])

            # -------- P6: ct GAT -> tok_cons --------------------------------
            if PH < 6: break
            with tc.tile_pool(name=f"p6_{it}", bufs=2) as sb6, \
                 tc.tile_pool(name=f"p6pv_{it}", bufs=2, space="PSUM") as ps6a, \
                 tc.tile_pool(name=f"p6ps_{it}", bufs=2, space="PSUM") as ps6b:

                tkw_tiles = {}

                def wb_tok(w, hh, pv, rec, sb):
                    if hh == 0:
                        tkw_tiles[w] = sb.tile([128, DH], f32, name="tkw")
                    tkw = tkw_tiles[w]
                    nc.vector.tensor_tensor(
                        out=tkw[:, hh * 384:(hh + 1) * 384].rearrange("p (d h) -> p d h", h=8),
                        in0=pv[:, :384].rearrange("p (d h) -> p d h", h=8),
                        in1=rec[:].unsqueeze(1).broadcast_to([128, 48, 8]),
                        op=OP.mult)

                def wb_tok_fin(w, sb):
                    tkw = tkw_tiles.pop(w)
                    if it == NITER - 1:
                        nc.sync.dma_start(out=tcf_dram[w * 128:(w + 1) * 128, :], in_=tkw[:])
                    stg = sb.tile([128, DH], bf16, name="stgt")
                    nc.vector.tensor_copy(out=stg[:], in_=tkw[:])
                    nc.sync.dma_start(out=tkc_dram[w * 128:(w + 1) * 128, :], in_=stg[:])
                    if it == 0:
                        stg_nat = sb.tile([128, DH], bf16, name="stgn")
                        nc.vector.tensor_copy(
                            out=stg_nat[:].rearrange("p (h d) -> p h d", h=8),
                            in_=stg[:].rearrange("p (d h) -> p h d", h=8))
                        nc.sync.dma_start(out=tt_in[w * 128:(w + 1) * 128, :], in_=stg_nat[:])

                def wb_tok_fin2(w, sb):
                    wb_tok_fin(w, sb)
                    if it == 0 and NITER > 1 and w in (3, 7):
                        hof = (w // 4) * 512
                        nc.gpsimd.collective_compute(
                            "AllGather", OP.bypass, replica_groups=RG,
                            ins=[tt_in[hof:hof + 512, :]],
                            outs=[tt_tab[hof * NCORES:(hof + 512) * NCORES, :]])

                gat_pass(sb6, ps6a, ps6b, tk_tab, SWK, idx_ct_t, oh_ct_d, ohd_ct_d,
                         sd_q, WCT, C_CT, DH, blk0=0, writeback=wb_tok,
                         wb_final=wb_tok_fin2, tag="ct")

        # -------- P7: fuse gate + blend -------------------------------------
        if PH >= 7:
          with tc.tile_pool(name="p7", bufs=1) as sb7, \
              tc.tile_pool(name="p7s", bufs=3) as sb7s, \
              tc.tile_pool(name="p7p", bufs=2, space="PSUM") as ps7:
             fw_t = sb7.tile([128, 12, DH], bf16, name="fwt")
             nc.sync.dma_start(out=fw_t[:], in_=fuse_w_d[:].rearrange("(k p) n -> p k n", p=128))
             fb_t = sb7.tile([1, DH], bf16, name="fbt")
             nc.sync.dma_start(out=fb_t[:], in_=fb_row_d[:])
             tokT_t = sb7.tile([128, 6, NTL], bf16, name="tokTt")
             nc.sync.dma_start(out=tokT_t[:], in_=tokT[:].rearrange("(k p) e -> p k e", p=128))
             tokcT_f = sb7.tile([128, 6, NTL], bf16, name="tokcTf")
             nc.sync.dma_start_transpose(out=tokcT_f[:], in_=tkc_dram[:])
             for m in range(WCT):
                 f_t = sb7s.tile([128, DH], f32, name="f_t")
                 for (n0, nw) in [(0, 512), (512, 256)]:
                     pf = ps7.tile([128, 512], f32, name="pf")
                     for k in range(12):
                         lt = tokT_t[:, k, m * 128:(m + 1) * 128] if k < 6 else \
                              tokcT_f[:, k - 6, m * 128:(m + 1) * 128]
                         nc.tensor.matmul(out=pf[:, :nw], lhsT=lt, rhs=fw_t[:, k, n0:n0 + nw],
                                          start=(k == 0), stop=False)
                     nc.tensor.matmul(out=pf[:, :nw], lhsT=ones1_t[:], rhs=fb_t[:, n0:n0 + nw],
                                      start=False, stop=True)
                     nc.scalar.activation(out=f_t[:, n0:n0 + nw], in_=pf[:, :nw], func=AF.Sigmoid)
                 tok_in = sb7s.tile([128, DH], f32, name="tok_in")
                 nc.sync.dma_start(out=tok_in[:], in_=tok_f32_d[m * 128:(m + 1) * 128, :])
                 tcf_in = sb7s.tile([128, DH], f32, name="tcf_in")
                 nc.sync.dma_start(out=tcf_in[:], in_=tcf_dram[m * 128:(m + 1) * 128, :])
                 dlt = sb7s.tile([128, DH], f32, name="dlt")
                 nc.vector.tensor_tensor(out=dlt[:], in0=tok_in[:], in1=tcf_in[:], op=OP.subtract)
                 nc.vector.tensor_tensor(out=dlt[:], in0=f_t[:], in1=dlt[:], op=OP.mult)
                 nc.vector.tensor_tensor(out=dlt[:], in0=tcf_in[:], in1=dlt[:], op=OP.add)
                 udlt = sb7s.tile([128, DH], f32, name="udlt")
                 nc.vector.tensor_copy(
                     out=udlt[:].rearrange("p (h d) -> p h d", h=8),
                     in_=dlt[:].rearrange("p (d h) -> p h d", h=8))
                 nc.sync.dma_start(out=out_d[m * 128:(m + 1) * 128, :], in_=udlt[:])

    nc.finalize()
    return nc


# ------------------------------------------------------------------ driver --

def run(inputs, trace=False):
    in_maps, C_CC, C_CT = _host_prep(inputs)
    nc = _build_nc(C_CC, C_CT)
    res = run_bass_kernel_spmd(nc, in_maps, core_ids=list(range(NCORES)), trace=trace)
    out = np.concatenate([res.results[c]["out"] for c in range(NCORES)], axis=0)
    return out.astype(np.float32), res


def kernel(**inputs) -> np.ndarray:
    out, _ = run(inputs, trace=False)
    return out
